# revision 12
# baseline (speedup 1.0000x reference)
"""Trainium2 Bass kernel for nn_DecoderTrans (dense transformer decoder layer + vocab head).

Sharding: 8 cores = (batch b, half hf). Each core computes the full trunk for its
512 "own" tokens (queries) and the K/V context for the whole 1024-token sequence
of its batch element. Own tokens always occupy key slots [512, 1024) so the
program is uniform SPMD; per-core mask/bias DATA encodes the causal structure.
Activations are kept feature-major (x^T: [D, tokens]) throughout; weights are
shipped pre-transposed ([d_in, d_out]).
"""
import math
import os
import sys

sys.path.insert(0, "/opt/trn_rl_repo")

import numpy as np

import concourse.bass as bass
import concourse.tile as tile
from concourse import bacc, mybir
from concourse.bass import ts
from concourse.masks import make_identity

P = 128
D = 512
DC = D // P          # 4 feature chunks
T = 1024             # full sequence (keys)
TOWN = 512           # own tokens per core (queries), slots [512, 1024)
H = 8
DKH = 64             # head dim
V = 32000
VCH = 500            # vocab columns per matmul (fits PSUM bank, >=256 for f32r)
VG = 4               # vocab chunks per group
NVG = V // (VCH * VG)  # 16 groups
FFN = 2 * D
NEG = -30000.0
SQRT_D = math.sqrt(D)
PAD_ID = 0

F32 = mybir.dt.float32
F32R = mybir.dt.float32r
I32 = mybir.dt.int32
AF = mybir.ActivationFunctionType
OP = mybir.AluOpType

# matmul input dtype: float32r streams 4x faster than float32 on TRN2 PE.
# fp32r is a rounded format: every producer writing a matmul operand must
# declare its output float32r, so operand tiles/DRAM tensors use MF dtype.
MM_DT = F32R if os.environ.get("KMM", "r") == "r" else F32
MF = MM_DT


def _r(ap):
    return ap


# --------------------------------------------------------------------------
# program builder
# --------------------------------------------------------------------------

def build_module():
    nc = bacc.Bacc("TRN2", target_bir_lowering=False, debug=False)

    def din(name, shape, dt=F32):
        return nc.dram_tensor(name, shape, dt, kind="ExternalInput").ap()

    a = {}
    a["idx"] = din("idx", [T, 1], I32)
    a["emb"] = din("emb", [V, D])
    a["peT"] = din("peT", [DC, P, T])
    a["enc"] = din("enc", [T, D])
    a["masks"] = din("masks", [P, 4, TOWN])
    a["biasS"] = din("biasS", [P, 8])
    a["biasC"] = din("biasC", [P, 8])
    for nm in ("WqT", "WkT", "WvT", "Wo1T", "cWqT", "eWkT", "eWvT", "Wo2T"):
        a[nm] = din(nm, [D, D], MF)
    a["W1T"] = din("W1T", [D, FFN], MF)
    a["W2T"] = din("W2T", [FFN, D], MF)
    a["WoutT"] = din("WoutT", [D, V], MF)
    # per-partition bias columns [P, n_out_chunks]
    for nm in ("bq", "bk", "bo1", "cbq", "ebk", "bo2", "b2"):
        a[nm + "_c"] = din(nm + "_c", [P, DC])
    a["b1_c"] = din("b1_c", [P, FFN // P])
    # bias rows for row-major (V) projections
    a["bv_r"] = din("bv_r", [1, D], MF)
    a["ebv_r"] = din("ebv_r", [1, D], MF)
    # layernorm gain/bias packs: [2, D] rows (g, b) and per-partition gains [P, DC]
    for i in (1, 2, 3):
        a[f"gb{i}"] = din(f"gb{i}", [2, D], MF)
        a[f"gc{i}"] = din(f"gc{i}", [P, DC])
    out = nc.dram_tensor("out", [TOWN, V], F32, kind="ExternalOutput").ap()
    a["out"] = out

    with tile.TileContext(nc) as tc, \
         nc.allow_low_precision(reason="fp32r matmul operand pipeline"):
        _emit(tc, a)
    nc.compile()
    return nc


def _emit(tc, a):
    nc = tc.nc

    with tc.tile_pool(name="const", bufs=1) as cp, \
         tc.tile_pool(name="trunk", bufs=1) as trunkp:
        # ---- constants ----
        ident = cp.tile([P, P], F32, tag="ident")
        make_identity(nc, ident[:])
        zscr = cp.tile([P, TOWN], F32, tag="zscr")
        nc.vector.memset(zscr[:], 0.0)
        ones_col = cp.tile([P, 1], MF, tag="ones_col")
        nc.scalar.add(ones_col[:], zscr[:, 0:1], 1.0)
        ones_row = cp.tile([1, P], MF, tag="ones_row")
        nc.scalar.add(ones_row[:], zscr[0:1, 0:P], 1.0)
        eps_c = cp.tile([1, 1], F32, tag="eps_c")
        nc.vector.memset(eps_c[:], 1e-5)
        biasS = cp.tile([P, 8], F32, tag="biasS")
        nc.sync.dma_start(biasS[:], a["biasS"][:, :])
        biasC = cp.tile([P, 8], F32, tag="biasC")
        nc.sync.dma_start(biasC[:], a["biasC"][:, :])
        masks = cp.tile([P, 4, TOWN], F32, tag="masks")
        nc.sync.dma_start(masks[:], a["masks"][:, :, :])

        def load_bias_col(nm, nch):
            t = cp.tile([P, nch], F32, tag=nm)
            nc.sync.dma_start(t[:], a[nm][:, :])
            return t
        bq_c = load_bias_col("bq_c", DC)
        bk_c = load_bias_col("bk_c", DC)
        bo1_c = load_bias_col("bo1_c", DC)
        cbq_c = load_bias_col("cbq_c", DC)
        ebk_c = load_bias_col("ebk_c", DC)
        bo2_c = load_bias_col("bo2_c", DC)
        b2_c = load_bias_col("b2_c", DC)
        b1_c = load_bias_col("b1_c", FFN // P)

        def load_row(nm, n):
            t = cp.tile([1, n], MF, tag=nm)
            nc.sync.dma_start(t[:], a[nm][:, :])
            return t
        bv_r = load_row("bv_r", D)
        ebv_r = load_row("ebv_r", D)
        gb = {i: None for i in (1, 2, 3)}
        gc = {i: None for i in (1, 2, 3)}
        for i in (1, 2, 3):
            gb[i] = cp.tile([2, D], MF, tag=f"gb{i}", name=f"gb{i}")
            nc.sync.dma_start(gb[i][:], a[f"gb{i}"][:, :])
            gc[i] = load_bias_col(f"gc{i}", DC)

        # ---- long-lived trunk activations ----
        x1T = [trunkp.tile([P, TOWN], MF, tag=f"x1T{c}", name=f"x1T{c}") for c in range(DC)]
        x2T = [trunkp.tile([P, TOWN], MF, tag=f"x2T{c}", name=f"x2T{c}") for c in range(DC)]
        x3T = [trunkp.tile([P, TOWN], MF, tag=f"x3T{c}", name=f"x3T{c}") for c in range(DC)]

        # ================= shared helpers =================

        def proj_fm(dsts, srcs, w_name, bias_col, ntok, func=AF.Identity,
                    wpool=None, pp=None, n_in=DC):
            """dsts[m][:, :ntok] = func(W @ srcs + b); feature-major in/out."""
            w_sb = []
            for c in range(n_in):
                w = wpool.tile([P, len(dsts) * P], MF, tag="w")
                nc.sync.dma_start(w[:], a[w_name][ts(c, P), :])
                w_sb.append(w)
            nth = (ntok + 511) // 512
            for m in range(len(dsts)):
                for th in range(nth):
                    nt = min(512, ntok - th * 512)
                    ps = pp.tile([P, 512], F32, tag="proj")
                    for c in range(n_in):
                        nc.tensor.matmul(
                            ps[:, :nt],
                            lhsT=_r(w_sb[c][:, ts(m, P)]),
                            rhs=_r(srcs[c][:, th * 512: th * 512 + nt]),
                            start=(c == 0), stop=(c == n_in - 1))
                    nc.scalar.activation(
                        dsts[m][:, th * 512: th * 512 + nt], ps[:, :nt],
                        func, bias=bias_col[:, m: m + 1], scale=1.0)

        def vproj(vtiles, srcs, w_name, bias_row, wpool=None, pp=None):
            """Row-major V projection with interleaved ones columns.

            vtiles[t]: [P, H*65]; cols h*65..h*65+63 = V features of head h,
            col h*65+64 = 1.0 (softmax-denominator trick)."""
            w_sb = []
            for c in range(DC):
                w = wpool.tile([P, D], MF, tag="w")
                nc.sync.dma_start(w[:], a[w_name][ts(c, P), :])
                w_sb.append(w)
            for t in range(len(vtiles)):
                ps = pp.tile([P, D], F32, tag="vproj")
                for c in range(DC):
                    nc.tensor.matmul(ps[:], lhsT=_r(srcs[c][:, ts(t, P)]),
                                     rhs=_r(w_sb[c][:]),
                                     start=(c == 0), stop=False)
                nc.tensor.matmul(ps[:], lhsT=_r(ones_row[:]), rhs=_r(bias_row[:]),
                                 start=False, stop=True)
                vt = vtiles[t]
                v3 = vt[:].rearrange("p (h e) -> p h e", e=65)
                nc.scalar.copy(v3[:, :, 0:64],
                               ps[:].rearrange("p (h e) -> p h e", e=64))
                nc.scalar.add(v3[:, :, 64:65],
                              zscr[:, 0:8].rearrange("p (h e) -> p h e", e=1), 1.0)

        def attention(kT, vtiles, qT, bias_col, use_masks, mergedT, pools):
            sp, avp, rp, sbp = pools
            for h in range(H):
                hc, off = h // 2, (h % 2) * DKH
                av = avp.tile([DKH + 1, TOWN], F32, tag="av")
                for kc in range(8):
                    s = sp.tile([P, TOWN], F32, tag="s")
                    nc.tensor.matmul(
                        s[:], lhsT=_r(kT[hc][off:off + DKH, ts(kc, P)]),
                        rhs=_r(qT[hc][off:off + DKH, :]), start=True, stop=True)
                    if use_masks and kc >= 4:
                        nc.vector.tensor_tensor(s[:], s[:], masks[:, kc - 4, :],
                                                op=OP.add)
                    pt = sbp.tile([P, TOWN], MF, tag="pT")
                    nc.scalar.activation(pt[:], s[:], AF.Exp,
                                         bias=bias_col[:, kc: kc + 1], scale=0.125)
                    nc.tensor.matmul(av[:],
                                     lhsT=_r(vtiles[kc][:, h * 65: h * 65 + 65]),
                                     rhs=_r(pt[:]), start=(kc == 0), stop=(kc == 7))
                rrow = sbp.tile([1, TOWN], MF, tag="rrow", bufs=2)
                nc.vector.reciprocal(rrow[:], av[DKH: DKH + 1, :])
                R = rp.tile([DKH, TOWN], F32, tag="R")
                nc.tensor.matmul(R[:], lhsT=_r(ones_row[:, 0:DKH]), rhs=_r(rrow[:]),
                                 start=True, stop=True)
                avs = sbp.tile([DKH, TOWN], F32, tag="avs", bufs=2)
                nc.scalar.copy(avs[:], av[0:DKH, :])
                nc.vector.tensor_tensor(mergedT[hc][off:off + DKH, :],
                                        avs[:], R[:], op=OP.mult)

        def layernorm(srcs, i, dsts, pools):
            """dsts = LN(srcs) with gain/bias pack i (feature-major chunks)."""
            statp, bcp, sbp = pools
            ssum = statp.tile([1, TOWN], F32, tag="ssum")
            ssq = statp.tile([1, TOWN], F32, tag="ssq")
            for c in range(DC):
                nc.tensor.matmul(ssum[:], lhsT=_r(ones_col[:]), rhs=_r(srcs[c][:]),
                                 start=(c == 0), stop=(c == DC - 1))
            for c in range(DC):
                sq = sbp.tile([P, TOWN], MF, tag="sq", bufs=2)
                nc.scalar.square(sq[:], srcs[c][:])
                nc.tensor.matmul(ssq[:], lhsT=_r(ones_col[:]), rhs=_r(sq[:]),
                                 start=(c == 0), stop=(c == DC - 1))
            mu = sbp.tile([1, TOWN], F32, tag="mu", bufs=1)
            nc.scalar.mul(mu[:], ssum[:], 1.0 / D)
            ex2 = sbp.tile([1, TOWN], F32, tag="ex2", bufs=1)
            nc.scalar.mul(ex2[:], ssq[:], 1.0 / D)
            musq = sbp.tile([1, TOWN], F32, tag="musq", bufs=1)
            nc.scalar.square(musq[:], mu[:])
            var = sbp.tile([1, TOWN], F32, tag="var", bufs=1)
            nc.vector.scalar_tensor_tensor(var[:], in0=musq[:], scalar=-1.0,
                                           in1=ex2[:], op0=OP.mult, op1=OP.add)
            std = sbp.tile([1, TOWN], F32, tag="std", bufs=1)
            nc.scalar.activation(std[:], var[:], AF.Sqrt, bias=eps_c[:], scale=1.0)
            arow = sbp.tile([1, TOWN], MF, tag="arow", bufs=1)
            nc.vector.reciprocal(arow[:], std[:])
            crow = sbp.tile([2, TOWN], MF, tag="crow", bufs=1)
            nc.scalar.add(crow[:, :], zscr[0:2, :], 1.0)
            nc.vector.scalar_tensor_tensor(crow[0:1, :], in0=mu[:], scalar=-1.0,
                                           in1=arow[:], op0=OP.mult, op1=OP.mult)
            A = bcp.tile([P, TOWN], F32, tag="A", bufs=1)
            nc.tensor.matmul(A[:], lhsT=_r(ones_row[:]), rhs=_r(arow[:]),
                             start=True, stop=True)
            for c in range(DC):
                C = bcp.tile([P, TOWN], F32, tag="C")
                nc.tensor.matmul(C[:], lhsT=_r(gb[i][:, ts(c, P)]), rhs=_r(crow[:]),
                                 start=True, stop=True)
                tmp = sbp.tile([P, TOWN], F32, tag="lnt", bufs=2)
                nc.vector.tensor_tensor(tmp[:], srcs[c][:], A[:], op=OP.mult)
                nc.vector.scalar_tensor_tensor(
                    dsts[c][:], in0=tmp[:], scalar=gc[i][:, c: c + 1], in1=C[:],
                    op0=OP.mult, op1=OP.add)

        # ================= block A: embed, self-attention, LN1 =================
        with tc.tile_pool(name="blkA", bufs=1) as bA, \
             tc.tile_pool(name="rotA", bufs=3) as rA:
            x0T = [bA.tile([P, T], MF, tag=f"x0T{c}", name=f"x0T{c}") for c in range(DC)]
            kT = [bA.tile([P, T], MF, tag=f"kT{c}", name=f"kT{c}") for c in range(DC)]
            vsb = [bA.tile([P, H * 65], MF, tag=f"v{t}", name=f"v{t}") for t in range(8)]
            qT = [bA.tile([P, TOWN], MF, tag=f"qT{c}", name=f"qT{c}") for c in range(DC)]
            mergedT = [bA.tile([P, TOWN], MF, tag=f"mgT{c}", name=f"mgT{c}") for c in range(DC)]

            # --- embedding gather + transpose + scale + positional encoding ---
            with tc.tile_pool(name="pe", bufs=1) as pep, \
                 tc.tile_pool(name="psA0", bufs=3, space="PSUM") as pp0:
                idx_sb = pep.tile([P, 8], I32, tag="idx")
                nc.sync.dma_start(
                    idx_sb[:], a["idx"].rearrange("(c p) o -> p (c o)", p=P))
                peT_sb = [pep.tile([P, T], F32, tag=f"pe{c}", name=f"pe{c}") for c in range(DC)]
                for c in range(DC):
                    nc.sync.dma_start(peT_sb[c][:], a["peT"][c, :, :])
                for t in range(8):
                    xg = rA.tile([P, D], F32, tag="xg")
                    nc.gpsimd.indirect_dma_start(
                        out=xg[:], out_offset=None, in_=a["emb"][:, :],
                        in_offset=bass.IndirectOffsetOnAxis(
                            ap=idx_sb[:, t: t + 1], axis=0))
                    for c in range(DC):
                        tp = pp0.tile([P, P], F32, tag="tp")
                        nc.tensor.transpose(tp[:], xg[:, ts(c, P)], ident[:])
                        nc.vector.scalar_tensor_tensor(
                            x0T[c][:, ts(t, P)], in0=tp[:], scalar=SQRT_D,
                            in1=peT_sb[c][:, ts(t, P)], op0=OP.mult, op1=OP.add)

            # --- K, V, Q projections ---
            with tc.tile_pool(name="wA", bufs=8) as wp, \
                 tc.tile_pool(name="psA1", bufs=3, space="PSUM") as pp1:
                proj_fm(kT, x0T, "WkT", bk_c, T, wpool=wp, pp=pp1)
                vproj(vsb, x0T, "WvT", bv_r, wpool=wp, pp=pp1)
                proj_fm(qT, [x0T[c][:, 512:1024] for c in range(DC)],
                        "WqT", bq_c, TOWN, wpool=wp, pp=pp1)

            # --- causal self-attention ---
            with tc.tile_pool(name="psS", bufs=3, space="PSUM") as sp, \
                 tc.tile_pool(name="psAV", bufs=2, space="PSUM") as avp, \
                 tc.tile_pool(name="psR", bufs=2, space="PSUM") as rp, \
                 tc.tile_pool(name="sbA", bufs=3) as sbp:
                attention(kT, vsb, qT, biasS, True, mergedT, (sp, avp, rp, sbp))

            # --- Wo1 + residual + LN1 -> x1T ---
            with tc.tile_pool(name="wA2", bufs=4) as wp, \
                 tc.tile_pool(name="psA2", bufs=2, space="PSUM") as pp2, \
                 tc.tile_pool(name="psStat", bufs=1, space="PSUM") as statp, \
                 tc.tile_pool(name="psBC", bufs=2, space="PSUM") as bcp, \
                 tc.tile_pool(name="sbLN", bufs=3) as sbp:
                w_sb = []
                for c in range(DC):
                    w = wp.tile([P, D], MF, tag="w")
                    nc.sync.dma_start(w[:], a["Wo1T"][ts(c, P), :])
                    w_sb.append(w)
                ln_in = []
                for m in range(DC):
                    ps = pp2.tile([P, TOWN], F32, tag="proj")
                    for c in range(DC):
                        nc.tensor.matmul(ps[:], lhsT=_r(w_sb[c][:, ts(m, P)]),
                                         rhs=_r(mergedT[c][:]),
                                         start=(c == 0), stop=(c == DC - 1))
                    li = sbp.tile([P, TOWN], MF, tag=f"li{m}", name=f"li{m}", bufs=1)
                    nc.vector.scalar_tensor_tensor(
                        li[:], in0=ps[:], scalar=bo1_c[:, m: m + 1],
                        in1=x0T[m][:, 512:1024], op0=OP.add, op1=OP.add)
                    ln_in.append(li)
                layernorm(ln_in, 1, x1T, (statp, bcp, sbp))

        # ================= block B: cross-attention, LN2 =================
        with tc.tile_pool(name="blkB", bufs=1) as bB, \
             tc.tile_pool(name="rotB", bufs=3) as rB:
            encT = [bB.tile([P, T], MF, tag=f"encT{c}", name=f"encT{c}") for c in range(DC)]
            ekT = [bB.tile([P, T], MF, tag=f"ekT{c}", name=f"ekT{c}") for c in range(DC)]
            evsb = [bB.tile([P, H * 65], MF, tag=f"ev{t}", name=f"ev{t}") for t in range(8)]
            cqT = [bB.tile([P, TOWN], MF, tag=f"cqT{c}", name=f"cqT{c}") for c in range(DC)]
            mergedT2 = [bB.tile([P, TOWN], MF, tag=f"mg2T{c}", name=f"mg2T{c}") for c in range(DC)]

            with tc.tile_pool(name="psB0", bufs=3, space="PSUM") as pp0:
                for t in range(8):
                    es = rB.tile([P, D], F32, tag="es")
                    nc.sync.dma_start(es[:], a["enc"][ts(t, P), :])
                    for c in range(DC):
                        tp = pp0.tile([P, P], F32, tag="tp")
                        nc.tensor.transpose(tp[:], es[:, ts(c, P)], ident[:])
                        nc.scalar.copy(encT[c][:, ts(t, P)], tp[:])

            with tc.tile_pool(name="wB", bufs=8) as wp, \
                 tc.tile_pool(name="psB1", bufs=3, space="PSUM") as pp1:
                proj_fm(ekT, encT, "eWkT", ebk_c, T, wpool=wp, pp=pp1)
                vproj(evsb, encT, "eWvT", ebv_r, wpool=wp, pp=pp1)
                proj_fm(cqT, x1T, "cWqT", cbq_c, TOWN, wpool=wp, pp=pp1)

            with tc.tile_pool(name="psS", bufs=3, space="PSUM") as sp, \
                 tc.tile_pool(name="psAV", bufs=2, space="PSUM") as avp, \
                 tc.tile_pool(name="psR", bufs=2, space="PSUM") as rp, \
                 tc.tile_pool(name="sbB", bufs=3) as sbp:
                attention(ekT, evsb, cqT, biasC, False, mergedT2,
                          (sp, avp, rp, sbp))

            with tc.tile_pool(name="wB2", bufs=4) as wp, \
                 tc.tile_pool(name="psB2", bufs=2, space="PSUM") as pp2, \
                 tc.tile_pool(name="psStat", bufs=1, space="PSUM") as statp, \
                 tc.tile_pool(name="psBC", bufs=2, space="PSUM") as bcp, \
                 tc.tile_pool(name="sbLN", bufs=3) as sbp:
                w_sb = []
                for c in range(DC):
                    w = wp.tile([P, D], MF, tag="w")
                    nc.sync.dma_start(w[:], a["Wo2T"][ts(c, P), :])
                    w_sb.append(w)
                ln_in = []
                for m in range(DC):
                    ps = pp2.tile([P, TOWN], F32, tag="proj")
                    for c in range(DC):
                        nc.tensor.matmul(ps[:], lhsT=_r(w_sb[c][:, ts(m, P)]),
                                         rhs=_r(mergedT2[c][:]),
                                         start=(c == 0), stop=(c == DC - 1))
                    li = sbp.tile([P, TOWN], MF, tag=f"li{m}", name=f"li{m}", bufs=1)
                    nc.vector.scalar_tensor_tensor(
                        li[:], in0=ps[:], scalar=bo2_c[:, m: m + 1],
                        in1=x1T[m][:], op0=OP.add, op1=OP.add)
                    ln_in.append(li)
                layernorm(ln_in, 2, x2T, (statp, bcp, sbp))

        # ================= block C: FFN, LN3 =================
        with tc.tile_pool(name="wC", bufs=4) as wp1, \
             tc.tile_pool(name="wC2", bufs=8) as wp2, \
             tc.tile_pool(name="hC", bufs=1) as hp, \
             tc.tile_pool(name="psC", bufs=3, space="PSUM") as pp, \
             tc.tile_pool(name="psStat", bufs=1, space="PSUM") as statp, \
             tc.tile_pool(name="psBC", bufs=2, space="PSUM") as bcp, \
             tc.tile_pool(name="sbC", bufs=3) as sbp:
            hT = [hp.tile([P, TOWN], MF, tag=f"hT{m}", name=f"hT{m}") for m in range(FFN // P)]
            proj_fm(hT, x2T, "W1T", b1_c, TOWN, func=AF.Relu, wpool=wp1, pp=pp)
            w_sb = []
            for c in range(FFN // P):
                w = wp2.tile([P, D], MF, tag="w2")
                nc.sync.dma_start(w[:], a["W2T"][ts(c, P), :])
                w_sb.append(w)
            ln_in = []
            for m in range(DC):
                ps = pp.tile([P, TOWN], F32, tag="proj")
                for c in range(FFN // P):
                    nc.tensor.matmul(ps[:], lhsT=_r(w_sb[c][:, ts(m, P)]),
                                     rhs=_r(hT[c][:]),
                                     start=(c == 0), stop=(c == FFN // P - 1))
                li = sbp.tile([P, TOWN], MF, tag=f"li{m}", name=f"li{m}", bufs=1)
                nc.vector.scalar_tensor_tensor(
                    li[:], in0=ps[:], scalar=b2_c[:, m: m + 1], in1=x2T[m][:],
                    op0=OP.add, op1=OP.add)
                ln_in.append(li)
            layernorm(ln_in, 3, x3T, (statp, bcp, sbp))

        # ================= block D: vocab projection =================
        with tc.tile_pool(name="wD", bufs=8) as wp, \
             tc.tile_pool(name="stD", bufs=4) as stp, \
             tc.tile_pool(name="psD", bufs=2, space="PSUM") as pp:
            for vg in range(NVG):
                w_sb = []
                for c in range(DC):
                    w = wp.tile([P, VG * VCH], MF, tag="wo")
                    nc.sync.dma_start(
                        w[:], a["WoutT"][ts(c, P),
                                         vg * VG * VCH:(vg + 1) * VG * VCH])
                    w_sb.append(w)
                for t in range(TOWN // P):
                    ps = pp.tile([P, VG, 512], F32, tag="vps")
                    for j in range(VG):
                        for c in range(DC):
                            nc.tensor.matmul(
                                ps[:, j, 0:VCH],
                                lhsT=_r(x3T[c][:, ts(t, P)]),
                                rhs=_r(w_sb[c][:, ts(j, VCH)]),
                                start=(c == 0), stop=(c == DC - 1))
                    stage = stp.tile([P, VG * VCH], F32, tag="stage")
                    st3 = stage[:].rearrange("p (j e) -> p j e", e=VCH)
                    if t % 2 == 0:
                        nc.scalar.copy(st3, ps[:, :, 0:VCH])
                    else:
                        nc.vector.tensor_copy(st3, ps[:, :, 0:VCH])
                    nc.sync.dma_start(
                        a["out"][ts(t, P), vg * VG * VCH:(vg + 1) * VG * VCH],
                        stage[:])


# --------------------------------------------------------------------------
# host-side input preparation
# --------------------------------------------------------------------------

def _pos_encoding_np(t, d):
    pos = np.arange(t, dtype=np.float32)[:, None]
    freqs = 1.0 / (10000.0 ** (np.arange(0, d, 2, dtype=np.float32) / d))
    pe = np.zeros((t, d), np.float32)
    pe[:, 0::2] = np.sin(pos * freqs)
    pe[:, 1::2] = np.cos(pos * freqs)
    return pe


def _col_pack(b):
    """[n] -> [P, n//P] with element (p, c) = b[c*P + p]."""
    b = np.asarray(b, np.float32)
    return np.ascontiguousarray(b.reshape(-1, P).T)


def prep_in_maps(inputs):
    gi = lambda n: np.asarray(inputs[n])
    tokens = gi("tokens").astype(np.int32)                      # [4, 1024]
    enc_all = np.ascontiguousarray(gi("enc_embeddings").astype(np.float32))
    enc_pad = gi("enc_pad_mask").astype(bool)
    emb = np.ascontiguousarray(gi("emb").astype(np.float32))

    shared = {"emb": emb}
    for nm in ("Wq", "Wk", "Wv", "Wo1", "cWq", "eWk", "eWv", "Wo2", "W1", "W2",
               "Wout"):
        shared[nm + "T"] = np.ascontiguousarray(
            gi(nm).astype(np.float32).T)
    for nm, src in (("bq", "bq"), ("bk", "bk"), ("bo1", "bo1"), ("cbq", "cbq"),
                    ("ebk", "ebk"), ("bo2", "bo2"), ("b2", "b2"), ("b1", "b1")):
        shared[nm + "_c"] = _col_pack(gi(src))
    shared["bv_r"] = gi("bv").astype(np.float32).reshape(1, D)
    shared["ebv_r"] = gi("ebv").astype(np.float32).reshape(1, D)
    for i, (g, b) in ((1, ("g1", "be1")), (2, ("g3", "be3")), (3, ("g2", "be2"))):
        shared[f"gb{i}"] = np.ascontiguousarray(
            np.stack([gi(g).astype(np.float32), gi(b).astype(np.float32)]))
        shared[f"gc{i}"] = _col_pack(gi(g))

    # causal diagonal-block masks for key chunks 4..7 (slot space)
    kk = np.arange(P)[:, None]
    qq = np.arange(TOWN)[None, :]
    masks = np.zeros((P, 4, TOWN), np.float32)
    for j in range(4):
        masks[:, j, :] = np.where((j * P + kk) > qq, NEG, 0.0)
    shared["masks"] = masks

    pe = _pos_encoding_np(T, D)

    in_maps = []
    for core in range(8):
        b, hf = core // 2, core % 2
        own = tokens[b, hf * 512:(hf + 1) * 512]
        idx_full = np.concatenate([tokens[b, :512], own])        # [1024]
        pe_slots = np.concatenate([pe[:512], pe[hf * 512:(hf + 1) * 512]], axis=0)
        peT = np.ascontiguousarray(
            pe_slots.T.reshape(DC, P, T, order="C"))             # careful below
        # pe_slots.T is [D, T]; reshape to [DC, P, T] splits D into chunks
        biasS = np.where(idx_full == PAD_ID, NEG, 0.0).astype(np.float32)
        if hf == 0:
            biasS[:512] = NEG                                    # no prefix half
        biasC = np.where(enc_pad[b], NEG, 0.0).astype(np.float32)
        m = dict(shared)
        m["idx"] = np.ascontiguousarray(idx_full.reshape(T, 1))
        m["peT"] = peT
        m["enc"] = np.ascontiguousarray(enc_all[b])
        m["biasS"] = np.ascontiguousarray(biasS.reshape(8, P).T)
        m["biasC"] = np.ascontiguousarray(biasC.reshape(8, P).T)
        in_maps.append(m)
    return in_maps


def assemble(results, inputs):
    full = np.empty((4, 1024, V), np.float32)
    for core in range(8):
        b, hf = core // 2, core % 2
        full[b, hf * 512:(hf + 1) * 512] = results[core]["out"]
    bout = np.asarray(inputs["bout"], np.float32)
    if np.any(bout):
        full += bout[None, None, :]
    return full


# --------------------------------------------------------------------------
# public entry point
# --------------------------------------------------------------------------

def kernel(**inputs):
    from concourse.bass_utils import run_bass_kernel_spmd
    nc = build_module()
    in_maps = prep_in_maps(inputs)
    res = run_bass_kernel_spmd(nc, in_maps, core_ids=list(range(8)))
    return assemble(res.results, inputs)


if __name__ == "__main__":
    nc = build_module()
    print("built ok")


# revision 14
# speedup vs baseline: 1.0092x; 1.0092x over previous
"""Trainium2 Bass kernel for nn_DecoderTrans (dense transformer decoder layer + vocab head).

Sharding: 8 cores = (batch b, half hf). Each core computes the full trunk for its
512 "own" tokens (queries) and the K/V context for the whole 1024-token sequence
of its batch element. Own tokens always occupy key slots [512, 1024) so the
program is uniform SPMD; per-core mask/bias DATA encodes the causal structure.
Activations are kept feature-major (x^T: [D, tokens]) throughout; weights are
shipped pre-transposed ([d_in, d_out]).
"""
import math
import os
import sys

sys.path.insert(0, "/opt/trn_rl_repo")

import numpy as np

import concourse.bass as bass
import concourse.tile as tile
from concourse import bacc, mybir
from concourse.bass import ts
from concourse.masks import make_identity

P = 128
D = 512
DC = D // P          # 4 feature chunks
T = 1024             # full sequence (keys)
TOWN = 512           # own tokens per core (queries), slots [512, 1024)
H = 8
DKH = 64             # head dim
V = 32000
VCH = 500            # vocab columns per matmul (fits PSUM bank, >=256 for f32r)
VG = 4               # vocab chunks per group
NVG = V // (VCH * VG)  # 16 groups
FFN = 2 * D
NEG = -30000.0
SQRT_D = math.sqrt(D)
PAD_ID = 0

F32 = mybir.dt.float32
F32R = mybir.dt.float32r
I32 = mybir.dt.int32
AF = mybir.ActivationFunctionType
OP = mybir.AluOpType

# matmul input dtype: float32r streams 4x faster than float32 on TRN2 PE.
# fp32r is a rounded format: every producer writing a matmul operand must
# declare its output float32r, so operand tiles/DRAM tensors use MF dtype.
MM_DT = F32R if os.environ.get("KMM", "r") == "r" else F32
MF = MM_DT


def _r(ap):
    return ap


# --------------------------------------------------------------------------
# program builder
# --------------------------------------------------------------------------

def build_module():
    nc = bacc.Bacc("TRN2", target_bir_lowering=False, debug=False)

    def din(name, shape, dt=F32):
        return nc.dram_tensor(name, shape, dt, kind="ExternalInput").ap()

    a = {}
    a["idx"] = din("idx", [T, 1], I32)
    a["emb"] = din("emb", [V, D])
    a["peT"] = din("peT", [DC, P, T])
    a["enc"] = din("enc", [T, D])
    a["masks"] = din("masks", [P, 4, TOWN], MF)
    a["biasS"] = din("biasS", [P, 8])
    a["biasC"] = din("biasC", [P, 8])
    for nm in ("WqT", "WkT", "WvT", "Wo1T", "cWqT", "eWkT", "eWvT", "Wo2T"):
        a[nm] = din(nm, [D, D], MF)
    a["W1T"] = din("W1T", [D, FFN], MF)
    a["W2T"] = din("W2T", [FFN, D], MF)
    a["WoutT"] = din("WoutT", [D, V], MF)
    # per-partition bias columns [P, n_out_chunks]
    for nm in ("bq", "bk", "bo1", "cbq", "ebk", "bo2", "b2"):
        a[nm + "_c"] = din(nm + "_c", [P, DC])
    a["b1_c"] = din("b1_c", [P, FFN // P])
    # bias rows for row-major (V) projections
    a["bv_r"] = din("bv_r", [1, D], MF)
    a["ebv_r"] = din("ebv_r", [1, D], MF)
    # layernorm gain/bias per-partition packs [P, DC]
    for i in (1, 2, 3):
        a[f"gc{i}"] = din(f"gc{i}", [P, DC])
        a[f"bc{i}"] = din(f"bc{i}", [P, DC])
    out = nc.dram_tensor("out", [TOWN, V], F32, kind="ExternalOutput").ap()
    a["out"] = out

    with tile.TileContext(nc) as tc, \
         nc.allow_low_precision(reason="fp32r matmul operand pipeline"):
        _emit(tc, a)
    nc.compile()
    return nc


def _emit(tc, a):
    nc = tc.nc

    with tc.tile_pool(name="const", bufs=1) as cp, \
         tc.tile_pool(name="trunk", bufs=1) as trunkp:
        # ---- constants ----
        ident = cp.tile([P, P], F32, tag="ident")
        make_identity(nc, ident[:])
        ident_r = cp.tile([P, P], MF, tag="ident_r")
        nc.scalar.copy(ident_r[:], ident[:])
        zscr = cp.tile([P, TOWN], F32, tag="zscr")
        nc.vector.memset(zscr[:], 0.0)
        ones_col = cp.tile([P, 1], MF, tag="ones_col")
        nc.scalar.add(ones_col[:], zscr[:, 0:1], 1.0)
        ones_row = cp.tile([1, P], MF, tag="ones_row")
        nc.scalar.add(ones_row[:], zscr[0:1, 0:P], 1.0)
        eps_c = cp.tile([1, 1], F32, tag="eps_c")
        nc.vector.memset(eps_c[:], 1e-5)
        biasS = cp.tile([P, 8], F32, tag="biasS")
        nc.sync.dma_start(biasS[:], a["biasS"][:, :])
        biasC = cp.tile([P, 8], F32, tag="biasC")
        nc.sync.dma_start(biasC[:], a["biasC"][:, :])
        masks = cp.tile([P, 4, TOWN], MF, tag="masks")
        nc.sync.dma_start(masks[:], a["masks"][:, :, :])

        def load_bias_col(nm, nch):
            t = cp.tile([P, nch], F32, tag=nm)
            nc.sync.dma_start(t[:], a[nm][:, :])
            return t
        bq_c = load_bias_col("bq_c", DC)
        bk_c = load_bias_col("bk_c", DC)
        bo1_c = load_bias_col("bo1_c", DC)
        cbq_c = load_bias_col("cbq_c", DC)
        ebk_c = load_bias_col("ebk_c", DC)
        bo2_c = load_bias_col("bo2_c", DC)
        b2_c = load_bias_col("b2_c", DC)
        b1_c = load_bias_col("b1_c", FFN // P)

        def load_row(nm, n):
            t = cp.tile([1, n], MF, tag=nm)
            nc.sync.dma_start(t[:], a[nm][:, :])
            return t
        bv_r = load_row("bv_r", D)
        ebv_r = load_row("ebv_r", D)
        gc = {i: None for i in (1, 2, 3)}
        bc = {i: None for i in (1, 2, 3)}
        for i in (1, 2, 3):
            gc[i] = load_bias_col(f"gc{i}", DC)
            bc[i] = load_bias_col(f"bc{i}", DC)

        # ---- long-lived trunk activations ----
        x1T = [trunkp.tile([P, TOWN], MF, tag=f"x1T{c}", name=f"x1T{c}") for c in range(DC)]
        x2T = [trunkp.tile([P, TOWN], MF, tag=f"x2T{c}", name=f"x2T{c}") for c in range(DC)]
        x3T = [trunkp.tile([P, TOWN], MF, tag=f"x3T{c}", name=f"x3T{c}") for c in range(DC)]

        # ================= shared helpers =================

        def proj_fm(dsts, srcs, w_name, bias_col, ntok, func=AF.Identity,
                    wpool=None, pp=None, n_in=DC):
            """dsts[m][:, :ntok] = func(W @ srcs + b); feature-major in/out."""
            w_sb = []
            for c in range(n_in):
                w = wpool.tile([P, len(dsts) * P], MF, tag="w")
                nc.sync.dma_start(w[:], a[w_name][ts(c, P), :])
                w_sb.append(w)
            nth = (ntok + 511) // 512
            for m in range(len(dsts)):
                for th in range(nth):
                    nt = min(512, ntok - th * 512)
                    ps = pp.tile([P, 512], F32, tag="proj")
                    for c in range(n_in):
                        nc.tensor.matmul(
                            ps[:, :nt],
                            lhsT=_r(w_sb[c][:, ts(m, P)]),
                            rhs=_r(srcs[c][:, th * 512: th * 512 + nt]),
                            start=(c == 0), stop=(c == n_in - 1))
                    nc.scalar.activation(
                        dsts[m][:, th * 512: th * 512 + nt], ps[:, :nt],
                        func, bias=bias_col[:, m: m + 1], scale=1.0)

        def vproj(vtiles, srcs, w_name, bias_row, wpool=None, pp=None):
            """Row-major V projection with interleaved ones columns.

            vtiles[t]: [P, H*65]; cols h*65..h*65+63 = V features of head h,
            col h*65+64 = 1.0 (softmax-denominator trick)."""
            w_sb = []
            for c in range(DC):
                w = wpool.tile([P, D], MF, tag="w")
                nc.sync.dma_start(w[:], a[w_name][ts(c, P), :])
                w_sb.append(w)
            for t in range(len(vtiles)):
                ps = pp.tile([P, D], F32, tag="vproj")
                for c in range(DC):
                    nc.tensor.matmul(ps[:], lhsT=_r(srcs[c][:, ts(t, P)]),
                                     rhs=_r(w_sb[c][:]),
                                     start=(c == 0), stop=False)
                nc.tensor.matmul(ps[:], lhsT=_r(ones_row[:]), rhs=_r(bias_row[:]),
                                 start=False, stop=True)
                vt = vtiles[t]
                v3 = vt[:].rearrange("p (h e) -> p h e", e=65)
                nc.scalar.copy(v3[:, :, 0:64],
                               ps[:].rearrange("p (h e) -> p h e", e=64))
                nc.scalar.add(v3[:, :, 64:65],
                              zscr[:, 0:8].rearrange("p (h e) -> p h e", e=1), 1.0)

        def attention(kT, vtiles, qT, bias_col, use_masks, mergedT, pools):
            sp, avp, rp, sbp = pools
            for h in range(H):
                hc, off = h // 2, (h % 2) * DKH
                av = avp.tile([DKH + 1, TOWN], F32, tag="av")
                for kc in range(8):
                    s = sp.tile([P, TOWN], F32, tag="s", bufs=4)
                    masked = use_masks and kc >= 4
                    nc.tensor.matmul(
                        s[:], lhsT=_r(kT[hc][off:off + DKH, ts(kc, P)]),
                        rhs=_r(qT[hc][off:off + DKH, :]), start=True,
                        stop=not masked)
                    if masked:
                        nc.tensor.matmul(s[:], lhsT=ident_r[:],
                                         rhs=masks[:, kc - 4, :],
                                         start=False, stop=True)
                    pt = sbp.tile([P, TOWN], MF, tag="pT", bufs=4)
                    nc.scalar.activation(pt[:], s[:], AF.Exp,
                                         bias=bias_col[:, kc: kc + 1], scale=0.125)
                    nc.tensor.matmul(av[:],
                                     lhsT=_r(vtiles[kc][:, h * 65: h * 65 + 65]),
                                     rhs=_r(pt[:]), start=(kc == 0), stop=(kc == 7))
                srow = sbp.tile([1, TOWN], MF, tag="srow", bufs=2)
                nc.scalar.copy(srow[:], av[DKH: DKH + 1, :])
                R = rp.tile([DKH, TOWN], F32, tag="R")
                nc.tensor.matmul(R[:], lhsT=_r(ones_row[:, 0:DKH]), rhs=_r(srow[:]),
                                 start=True, stop=True)
                rinv = sbp.tile([DKH, TOWN], F32, tag="rinv", bufs=2)
                nc.vector.reciprocal(rinv[:], R[:])
                nc.vector.tensor_tensor(mergedT[hc][off:off + DKH, :],
                                        av[0:DKH, :], rinv[:], op=OP.mult)

        def layernorm(srcs, i, dsts, pools):
            """dsts = LN(srcs) with gain/bias pack i (feature-major chunks)."""
            statp, bcp, sbp = pools
            ssum = statp.tile([1, TOWN], F32, tag="ssum")
            ssq = statp.tile([1, TOWN], F32, tag="ssq")
            for c in range(DC):
                nc.tensor.matmul(ssum[:], lhsT=_r(ones_col[:]), rhs=_r(srcs[c][:]),
                                 start=(c == 0), stop=(c == DC - 1))
            for c in range(DC):
                sq = sbp.tile([P, TOWN], MF, tag="sq", bufs=2)
                nc.scalar.square(sq[:], srcs[c][:])
                nc.tensor.matmul(ssq[:], lhsT=_r(ones_col[:]), rhs=_r(sq[:]),
                                 start=(c == 0), stop=(c == DC - 1))
            mu = sbp.tile([1, TOWN], MF, tag="mu", bufs=1)
            nc.scalar.mul(mu[:], ssum[:], 1.0 / D)
            ex2 = sbp.tile([1, TOWN], F32, tag="ex2", bufs=1)
            nc.scalar.mul(ex2[:], ssq[:], 1.0 / D)
            musq = sbp.tile([1, TOWN], F32, tag="musq", bufs=1)
            nc.scalar.square(musq[:], mu[:])
            var = sbp.tile([1, TOWN], F32, tag="var", bufs=1)
            nc.vector.scalar_tensor_tensor(var[:], in0=musq[:], scalar=-1.0,
                                           in1=ex2[:], op0=OP.mult, op1=OP.add)
            std = sbp.tile([1, TOWN], MF, tag="std", bufs=1)
            nc.scalar.activation(std[:], var[:], AF.Sqrt, bias=eps_c[:], scale=1.0)
            mu_b = bcp.tile([P, TOWN], F32, tag="mu_b", bufs=1)
            nc.tensor.matmul(mu_b[:], lhsT=_r(ones_row[:]), rhs=_r(mu[:]),
                             start=True, stop=True)
            std_b = bcp.tile([P, TOWN], F32, tag="std_b", bufs=1)
            nc.tensor.matmul(std_b[:], lhsT=_r(ones_row[:]), rhs=_r(std[:]),
                             start=True, stop=True)
            ainv = sbp.tile([P, TOWN], F32, tag="ainv", bufs=1)
            nc.vector.reciprocal(ainv[:], std_b[:])
            for c in range(DC):
                t1 = sbp.tile([P, TOWN], F32, tag="lnt", bufs=2)
                nc.vector.tensor_tensor(t1[:], srcs[c][:], mu_b[:], op=OP.subtract)
                t2 = sbp.tile([P, TOWN], F32, tag="lnt2", bufs=2)
                nc.vector.tensor_tensor(t2[:], t1[:], ainv[:], op=OP.mult)
                nc.vector.tensor_scalar(
                    dsts[c][:], t2[:], gc[i][:, c: c + 1], bc[i][:, c: c + 1],
                    op0=OP.mult, op1=OP.add)

        # ================= block A: embed, self-attention, LN1 =================
        with tc.tile_pool(name="blkA", bufs=1) as bA, \
             tc.tile_pool(name="rotA", bufs=3) as rA:
            x0T = [bA.tile([P, T], MF, tag=f"x0T{c}", name=f"x0T{c}") for c in range(DC)]
            kT = [bA.tile([P, T], MF, tag=f"kT{c}", name=f"kT{c}") for c in range(DC)]
            vsb = [bA.tile([P, H * 65], MF, tag=f"v{t}", name=f"v{t}") for t in range(8)]
            qT = [bA.tile([P, TOWN], MF, tag=f"qT{c}", name=f"qT{c}") for c in range(DC)]
            mergedT = [bA.tile([P, TOWN], MF, tag=f"mgT{c}", name=f"mgT{c}") for c in range(DC)]

            # --- embedding gather + transpose + scale + positional encoding ---
            with tc.tile_pool(name="pe", bufs=1) as pep, \
                 tc.tile_pool(name="psA0", bufs=3, space="PSUM") as pp0:
                idx_sb = pep.tile([P, 8], I32, tag="idx")
                nc.sync.dma_start(
                    idx_sb[:], a["idx"].rearrange("(c p) o -> p (c o)", p=P))
                peT_sb = [pep.tile([P, T], F32, tag=f"pe{c}", name=f"pe{c}") for c in range(DC)]
                for c in range(DC):
                    nc.sync.dma_start(peT_sb[c][:], a["peT"][c, :, :])
                for t in range(8):
                    xg = rA.tile([P, D], F32, tag="xg")
                    nc.gpsimd.indirect_dma_start(
                        out=xg[:], out_offset=None, in_=a["emb"][:, :],
                        in_offset=bass.IndirectOffsetOnAxis(
                            ap=idx_sb[:, t: t + 1], axis=0))
                    for c in range(DC):
                        tp = pp0.tile([P, P], F32, tag="tp")
                        nc.tensor.transpose(tp[:], xg[:, ts(c, P)], ident[:])
                        nc.vector.scalar_tensor_tensor(
                            x0T[c][:, ts(t, P)], in0=tp[:], scalar=SQRT_D,
                            in1=peT_sb[c][:, ts(t, P)], op0=OP.mult, op1=OP.add)

            # --- K, V, Q projections ---
            with tc.tile_pool(name="wA", bufs=8) as wp, \
                 tc.tile_pool(name="psA1", bufs=3, space="PSUM") as pp1:
                proj_fm(kT, x0T, "WkT", bk_c, T, wpool=wp, pp=pp1)
                vproj(vsb, x0T, "WvT", bv_r, wpool=wp, pp=pp1)
                proj_fm(qT, [x0T[c][:, 512:1024] for c in range(DC)],
                        "WqT", bq_c, TOWN, wpool=wp, pp=pp1)

            # --- causal self-attention ---
            with tc.tile_pool(name="psS", bufs=3, space="PSUM") as sp, \
                 tc.tile_pool(name="psAV", bufs=2, space="PSUM") as avp, \
                 tc.tile_pool(name="psR", bufs=2, space="PSUM") as rp, \
                 tc.tile_pool(name="sbA", bufs=3) as sbp:
                attention(kT, vsb, qT, biasS, True, mergedT, (sp, avp, rp, sbp))

            # --- Wo1 + residual + LN1 -> x1T ---
            with tc.tile_pool(name="wA2", bufs=4) as wp, \
                 tc.tile_pool(name="psA2", bufs=2, space="PSUM") as pp2, \
                 tc.tile_pool(name="psStat", bufs=1, space="PSUM") as statp, \
                 tc.tile_pool(name="psBC", bufs=2, space="PSUM") as bcp, \
                 tc.tile_pool(name="sbLN", bufs=3) as sbp:
                w_sb = []
                for c in range(DC):
                    w = wp.tile([P, D], MF, tag="w")
                    nc.sync.dma_start(w[:], a["Wo1T"][ts(c, P), :])
                    w_sb.append(w)
                ln_in = []
                for m in range(DC):
                    ps = pp2.tile([P, TOWN], F32, tag="proj")
                    for c in range(DC):
                        nc.tensor.matmul(ps[:], lhsT=_r(w_sb[c][:, ts(m, P)]),
                                         rhs=_r(mergedT[c][:]),
                                         start=(c == 0), stop=(c == DC - 1))
                    li = sbp.tile([P, TOWN], MF, tag=f"li{m}", name=f"li{m}", bufs=1)
                    nc.vector.scalar_tensor_tensor(
                        li[:], in0=ps[:], scalar=bo1_c[:, m: m + 1],
                        in1=x0T[m][:, 512:1024], op0=OP.add, op1=OP.add)
                    ln_in.append(li)
                layernorm(ln_in, 1, x1T, (statp, bcp, sbp))

        # ================= block B: cross-attention, LN2 =================
        with tc.tile_pool(name="blkB", bufs=1) as bB, \
             tc.tile_pool(name="rotB", bufs=3) as rB:
            encT = [bB.tile([P, T], MF, tag=f"encT{c}", name=f"encT{c}") for c in range(DC)]
            ekT = [bB.tile([P, T], MF, tag=f"ekT{c}", name=f"ekT{c}") for c in range(DC)]
            evsb = [bB.tile([P, H * 65], MF, tag=f"ev{t}", name=f"ev{t}") for t in range(8)]
            cqT = [bB.tile([P, TOWN], MF, tag=f"cqT{c}", name=f"cqT{c}") for c in range(DC)]
            mergedT2 = [bB.tile([P, TOWN], MF, tag=f"mg2T{c}", name=f"mg2T{c}") for c in range(DC)]

            with tc.tile_pool(name="psB0", bufs=3, space="PSUM") as pp0:
                for t in range(8):
                    es = rB.tile([P, D], F32, tag="es")
                    nc.sync.dma_start(es[:], a["enc"][ts(t, P), :])
                    for c in range(DC):
                        tp = pp0.tile([P, P], F32, tag="tp")
                        nc.tensor.transpose(tp[:], es[:, ts(c, P)], ident[:])
                        nc.scalar.copy(encT[c][:, ts(t, P)], tp[:])

            with tc.tile_pool(name="wB", bufs=8) as wp, \
                 tc.tile_pool(name="psB1", bufs=3, space="PSUM") as pp1:
                proj_fm(ekT, encT, "eWkT", ebk_c, T, wpool=wp, pp=pp1)
                vproj(evsb, encT, "eWvT", ebv_r, wpool=wp, pp=pp1)
                proj_fm(cqT, x1T, "cWqT", cbq_c, TOWN, wpool=wp, pp=pp1)

            with tc.tile_pool(name="psS", bufs=3, space="PSUM") as sp, \
                 tc.tile_pool(name="psAV", bufs=2, space="PSUM") as avp, \
                 tc.tile_pool(name="psR", bufs=2, space="PSUM") as rp, \
                 tc.tile_pool(name="sbB", bufs=3) as sbp:
                attention(ekT, evsb, cqT, biasC, False, mergedT2,
                          (sp, avp, rp, sbp))

            with tc.tile_pool(name="wB2", bufs=4) as wp, \
                 tc.tile_pool(name="psB2", bufs=2, space="PSUM") as pp2, \
                 tc.tile_pool(name="psStat", bufs=1, space="PSUM") as statp, \
                 tc.tile_pool(name="psBC", bufs=2, space="PSUM") as bcp, \
                 tc.tile_pool(name="sbLN", bufs=3) as sbp:
                w_sb = []
                for c in range(DC):
                    w = wp.tile([P, D], MF, tag="w")
                    nc.sync.dma_start(w[:], a["Wo2T"][ts(c, P), :])
                    w_sb.append(w)
                ln_in = []
                for m in range(DC):
                    ps = pp2.tile([P, TOWN], F32, tag="proj")
                    for c in range(DC):
                        nc.tensor.matmul(ps[:], lhsT=_r(w_sb[c][:, ts(m, P)]),
                                         rhs=_r(mergedT2[c][:]),
                                         start=(c == 0), stop=(c == DC - 1))
                    li = sbp.tile([P, TOWN], MF, tag=f"li{m}", name=f"li{m}", bufs=1)
                    nc.vector.scalar_tensor_tensor(
                        li[:], in0=ps[:], scalar=bo2_c[:, m: m + 1],
                        in1=x1T[m][:], op0=OP.add, op1=OP.add)
                    ln_in.append(li)
                layernorm(ln_in, 2, x2T, (statp, bcp, sbp))

        # ================= block C: FFN, LN3 =================
        with tc.tile_pool(name="wC", bufs=4) as wp1, \
             tc.tile_pool(name="wC2", bufs=8) as wp2, \
             tc.tile_pool(name="hC", bufs=1) as hp, \
             tc.tile_pool(name="psC", bufs=3, space="PSUM") as pp, \
             tc.tile_pool(name="psStat", bufs=1, space="PSUM") as statp, \
             tc.tile_pool(name="psBC", bufs=2, space="PSUM") as bcp, \
             tc.tile_pool(name="sbC", bufs=3) as sbp:
            hT = [hp.tile([P, TOWN], MF, tag=f"hT{m}", name=f"hT{m}") for m in range(FFN // P)]
            proj_fm(hT, x2T, "W1T", b1_c, TOWN, func=AF.Relu, wpool=wp1, pp=pp)
            w_sb = []
            for c in range(FFN // P):
                w = wp2.tile([P, D], MF, tag="w2")
                nc.sync.dma_start(w[:], a["W2T"][ts(c, P), :])
                w_sb.append(w)
            ln_in = []
            for m in range(DC):
                ps = pp.tile([P, TOWN], F32, tag="proj")
                for c in range(FFN // P):
                    nc.tensor.matmul(ps[:], lhsT=_r(w_sb[c][:, ts(m, P)]),
                                     rhs=_r(hT[c][:]),
                                     start=(c == 0), stop=(c == FFN // P - 1))
                li = sbp.tile([P, TOWN], MF, tag=f"li{m}", name=f"li{m}", bufs=1)
                nc.vector.scalar_tensor_tensor(
                    li[:], in0=ps[:], scalar=b2_c[:, m: m + 1], in1=x2T[m][:],
                    op0=OP.add, op1=OP.add)
                ln_in.append(li)
            layernorm(ln_in, 3, x3T, (statp, bcp, sbp))

        # ================= block D: vocab projection =================
        with tc.tile_pool(name="wD", bufs=8) as wp, \
             tc.tile_pool(name="stD", bufs=4) as stp, \
             tc.tile_pool(name="psD", bufs=2, space="PSUM") as pp:
            for vg in range(NVG):
                w_sb = []
                for c in range(DC):
                    w = wp.tile([P, VG * VCH], MF, tag="wo")
                    nc.sync.dma_start(
                        w[:], a["WoutT"][ts(c, P),
                                         vg * VG * VCH:(vg + 1) * VG * VCH])
                    w_sb.append(w)
                for t in range(TOWN // P):
                    ps = pp.tile([P, VG, 512], F32, tag="vps")
                    for j in range(VG):
                        for c in range(DC):
                            nc.tensor.matmul(
                                ps[:, j, 0:VCH],
                                lhsT=_r(x3T[c][:, ts(t, P)]),
                                rhs=_r(w_sb[c][:, ts(j, VCH)]),
                                start=(c == 0), stop=(c == DC - 1))
                    stage = stp.tile([P, VG * VCH], F32, tag="stage")
                    st3 = stage[:].rearrange("p (j e) -> p j e", e=VCH)
                    if t % 2 == 0:
                        nc.scalar.copy(st3, ps[:, :, 0:VCH])
                    else:
                        nc.vector.tensor_copy(st3, ps[:, :, 0:VCH])
                    nc.sync.dma_start(
                        a["out"][ts(t, P), vg * VG * VCH:(vg + 1) * VG * VCH],
                        stage[:])


# --------------------------------------------------------------------------
# host-side input preparation
# --------------------------------------------------------------------------

def _pos_encoding_np(t, d):
    pos = np.arange(t, dtype=np.float32)[:, None]
    freqs = 1.0 / (10000.0 ** (np.arange(0, d, 2, dtype=np.float32) / d))
    pe = np.zeros((t, d), np.float32)
    pe[:, 0::2] = np.sin(pos * freqs)
    pe[:, 1::2] = np.cos(pos * freqs)
    return pe


def _col_pack(b):
    """[n] -> [P, n//P] with element (p, c) = b[c*P + p]."""
    b = np.asarray(b, np.float32)
    return np.ascontiguousarray(b.reshape(-1, P).T)


def prep_in_maps(inputs):
    gi = lambda n: np.asarray(inputs[n])
    tokens = gi("tokens").astype(np.int32)                      # [4, 1024]
    enc_all = np.ascontiguousarray(gi("enc_embeddings").astype(np.float32))
    enc_pad = gi("enc_pad_mask").astype(bool)
    emb = np.ascontiguousarray(gi("emb").astype(np.float32))

    shared = {"emb": emb}
    for nm in ("Wq", "Wk", "Wv", "Wo1", "cWq", "eWk", "eWv", "Wo2", "W1", "W2",
               "Wout"):
        shared[nm + "T"] = np.ascontiguousarray(
            gi(nm).astype(np.float32).T)
    for nm, src in (("bq", "bq"), ("bk", "bk"), ("bo1", "bo1"), ("cbq", "cbq"),
                    ("ebk", "ebk"), ("bo2", "bo2"), ("b2", "b2"), ("b1", "b1")):
        shared[nm + "_c"] = _col_pack(gi(src))
    shared["bv_r"] = gi("bv").astype(np.float32).reshape(1, D)
    shared["ebv_r"] = gi("ebv").astype(np.float32).reshape(1, D)
    for i, (g, b) in ((1, ("g1", "be1")), (2, ("g3", "be3")), (3, ("g2", "be2"))):
        shared[f"gc{i}"] = _col_pack(gi(g))
        shared[f"bc{i}"] = _col_pack(gi(b))

    # causal diagonal-block masks for key chunks 4..7 (slot space)
    kk = np.arange(P)[:, None]
    qq = np.arange(TOWN)[None, :]
    masks = np.zeros((P, 4, TOWN), np.float32)
    for j in range(4):
        masks[:, j, :] = np.where((j * P + kk) > qq, NEG, 0.0)
    shared["masks"] = masks

    pe = _pos_encoding_np(T, D)

    in_maps = []
    for core in range(8):
        b, hf = core // 2, core % 2
        own = tokens[b, hf * 512:(hf + 1) * 512]
        idx_full = np.concatenate([tokens[b, :512], own])        # [1024]
        pe_slots = np.concatenate([pe[:512], pe[hf * 512:(hf + 1) * 512]], axis=0)
        peT = np.ascontiguousarray(
            pe_slots.T.reshape(DC, P, T, order="C"))             # careful below
        # pe_slots.T is [D, T]; reshape to [DC, P, T] splits D into chunks
        biasS = np.where(idx_full == PAD_ID, NEG, 0.0).astype(np.float32)
        if hf == 0:
            biasS[:512] = NEG                                    # no prefix half
        biasC = np.where(enc_pad[b], NEG, 0.0).astype(np.float32)
        m = dict(shared)
        m["idx"] = np.ascontiguousarray(idx_full.reshape(T, 1))
        m["peT"] = peT
        m["enc"] = np.ascontiguousarray(enc_all[b])
        m["biasS"] = np.ascontiguousarray(biasS.reshape(8, P).T)
        m["biasC"] = np.ascontiguousarray(biasC.reshape(8, P).T)
        in_maps.append(m)
    return in_maps


def assemble(results, inputs):
    full = np.empty((4, 1024, V), np.float32)
    for core in range(8):
        b, hf = core // 2, core % 2
        full[b, hf * 512:(hf + 1) * 512] = results[core]["out"]
    bout = np.asarray(inputs["bout"], np.float32)
    if np.any(bout):
        full += bout[None, None, :]
    return full


# --------------------------------------------------------------------------
# public entry point
# --------------------------------------------------------------------------

def kernel(**inputs):
    from concourse.bass_utils import run_bass_kernel_spmd
    nc = build_module()
    in_maps = prep_in_maps(inputs)
    res = run_bass_kernel_spmd(nc, in_maps, core_ids=list(range(8)))
    return assemble(res.results, inputs)


if __name__ == "__main__":
    nc = build_module()
    print("built ok")


# revision 16
# speedup vs baseline: 1.1947x; 1.1839x over previous
"""Trainium2 Bass kernel for nn_DecoderTrans (dense transformer decoder layer + vocab head).

Sharding: 8 cores = (batch b, half hf). Each core computes the full trunk for its
512 "own" tokens (queries) and the K/V context for the whole 1024-token sequence
of its batch element. Own tokens always occupy key slots [512, 1024) so the
program is uniform SPMD; per-core mask/bias DATA encodes the causal structure.
Activations are kept feature-major (x^T: [D, tokens]) throughout; weights are
shipped pre-transposed ([d_in, d_out]).
"""
import math
import os
import sys

sys.path.insert(0, "/opt/trn_rl_repo")

import numpy as np

import concourse.bass as bass
import concourse.tile as tile
from concourse import bacc, mybir
from concourse.bass import ts
from concourse.masks import make_identity

P = 128
D = 512
DC = D // P          # 4 feature chunks
T = 1024             # full sequence (keys)
TOWN = 512           # own tokens per core (queries), slots [512, 1024)
H = 8
DKH = 64             # head dim
V = 32000
VCH = 500            # vocab columns per matmul (fits PSUM bank, >=256 for f32r)
VG = 4               # vocab chunks per group
NVG = V // (VCH * VG)  # 16 groups
FFN = 2 * D
NEG = -30000.0
SQRT_D = math.sqrt(D)
PAD_ID = 0

F32 = mybir.dt.float32
F32R = mybir.dt.float32r
I32 = mybir.dt.int32
AF = mybir.ActivationFunctionType
OP = mybir.AluOpType

# matmul input dtype: float32r streams 4x faster than float32 on TRN2 PE.
# fp32r is a rounded format: every producer writing a matmul operand must
# declare its output float32r, so operand tiles/DRAM tensors use MF dtype.
MM_DT = F32R if os.environ.get("KMM", "r") == "r" else F32
MF = MM_DT
BF16 = mybir.dt.bfloat16
# vocab projection operand dtype: bf16 halves the Wout DMA stream (the vocab
# stage is HBM-bound); 'r' keeps fp32r end to end
WOUT_DT = BF16 if os.environ.get("KWOUT", "b") == "b" else MM_DT


def _r(ap):
    return ap


# --------------------------------------------------------------------------
# program builder
# --------------------------------------------------------------------------

def build_module():
    nc = bacc.Bacc("TRN2", target_bir_lowering=False, debug=False)

    def din(name, shape, dt=F32):
        return nc.dram_tensor(name, shape, dt, kind="ExternalInput").ap()

    a = {}
    a["idx"] = din("idx", [T, 1], I32)
    a["emb"] = din("emb", [V, D])
    a["peT"] = din("peT", [DC, P, T])
    a["enc"] = din("enc", [T, D])
    a["masks"] = din("masks", [P, 4, TOWN], MF)
    a["biasS"] = din("biasS", [P, 8])
    a["biasC"] = din("biasC", [P, 8])
    for nm in ("WqT", "WkT", "WvT", "Wo1T", "cWqT", "eWkT", "eWvT", "Wo2T"):
        a[nm] = din(nm, [D, D], MF)
    a["W1T"] = din("W1T", [D, FFN], MF)
    a["W2T"] = din("W2T", [FFN, D], MF)
    a["WoutT"] = din("WoutT", [D, V], WOUT_DT)
    # per-partition bias columns [P, n_out_chunks]
    for nm in ("bq", "bk", "bo1", "cbq", "ebk", "bo2", "b2"):
        a[nm + "_c"] = din(nm + "_c", [P, DC])
    a["b1_c"] = din("b1_c", [P, FFN // P])
    # bias rows for row-major (V) projections
    a["bv_r"] = din("bv_r", [1, D], MF)
    a["ebv_r"] = din("ebv_r", [1, D], MF)
    # layernorm gain/bias per-partition packs [P, DC]
    for i in (1, 2, 3):
        a[f"gc{i}"] = din(f"gc{i}", [P, DC])
        a[f"bc{i}"] = din(f"bc{i}", [P, DC])
    out = nc.dram_tensor("out", [TOWN, V], F32, kind="ExternalOutput").ap()
    a["out"] = out

    with tile.TileContext(nc) as tc, \
         nc.allow_low_precision(reason="fp32r matmul operand pipeline"):
        _emit(tc, a)
    nc.compile()
    return nc


def _emit(tc, a):
    nc = tc.nc

    with tc.tile_pool(name="const", bufs=1) as cp, \
         tc.tile_pool(name="trunk", bufs=1) as trunkp:
        # ---- constants ----
        ident = cp.tile([P, P], F32, tag="ident")
        make_identity(nc, ident[:])
        ident_r = cp.tile([P, P], MF, tag="ident_r")
        nc.scalar.copy(ident_r[:], ident[:])
        zscr = cp.tile([P, TOWN], F32, tag="zscr")
        nc.vector.memset(zscr[:], 0.0)
        ones_col = cp.tile([P, 1], MF, tag="ones_col")
        nc.scalar.add(ones_col[:], zscr[:, 0:1], 1.0)
        ones_row = cp.tile([1, P], MF, tag="ones_row")
        nc.scalar.add(ones_row[:], zscr[0:1, 0:P], 1.0)
        eps_c = cp.tile([1, 1], F32, tag="eps_c")
        nc.vector.memset(eps_c[:], 1e-5)
        biasS = cp.tile([P, 8], F32, tag="biasS")
        nc.sync.dma_start(biasS[:], a["biasS"][:, :])
        biasC = cp.tile([P, 8], F32, tag="biasC")
        nc.sync.dma_start(biasC[:], a["biasC"][:, :])
        masks = cp.tile([P, 4, TOWN], MF, tag="masks")
        nc.sync.dma_start(masks[:], a["masks"][:, :, :])

        def load_bias_col(nm, nch):
            t = cp.tile([P, nch], F32, tag=nm)
            nc.sync.dma_start(t[:], a[nm][:, :])
            return t
        bq_c = load_bias_col("bq_c", DC)
        bk_c = load_bias_col("bk_c", DC)
        bo1_c = load_bias_col("bo1_c", DC)
        cbq_c = load_bias_col("cbq_c", DC)
        ebk_c = load_bias_col("ebk_c", DC)
        bo2_c = load_bias_col("bo2_c", DC)
        b2_c = load_bias_col("b2_c", DC)
        b1_c = load_bias_col("b1_c", FFN // P)

        def load_row(nm, n):
            t = cp.tile([1, n], MF, tag=nm)
            nc.sync.dma_start(t[:], a[nm][:, :])
            return t
        bv_r = load_row("bv_r", D)
        ebv_r = load_row("ebv_r", D)
        gc = {i: None for i in (1, 2, 3)}
        bc = {i: None for i in (1, 2, 3)}
        for i in (1, 2, 3):
            gc[i] = load_bias_col(f"gc{i}", DC)
            bc[i] = load_bias_col(f"bc{i}", DC)

        # ---- long-lived trunk activations ----
        x1T = [trunkp.tile([P, TOWN], MF, tag=f"x1T{c}", name=f"x1T{c}") for c in range(DC)]
        x2T = [trunkp.tile([P, TOWN], MF, tag=f"x2T{c}", name=f"x2T{c}") for c in range(DC)]
        x3T = [trunkp.tile([P, TOWN], MF, tag=f"x3T{c}", name=f"x3T{c}") for c in range(DC)]

        # ================= shared helpers =================

        def proj_fm(dsts, srcs, w_name, bias_col, ntok, func=AF.Identity,
                    wpool=None, pp=None, n_in=DC):
            """dsts[m][:, :ntok] = func(W @ srcs + b); feature-major in/out."""
            w_sb = []
            for c in range(n_in):
                w = wpool.tile([P, len(dsts) * P], MF, tag="w")
                nc.sync.dma_start(w[:], a[w_name][ts(c, P), :])
                w_sb.append(w)
            nth = (ntok + 511) // 512
            for m in range(len(dsts)):
                for th in range(nth):
                    nt = min(512, ntok - th * 512)
                    ps = pp.tile([P, 512], F32, tag="proj")
                    for c in range(n_in):
                        nc.tensor.matmul(
                            ps[:, :nt],
                            lhsT=_r(w_sb[c][:, ts(m, P)]),
                            rhs=_r(srcs[c][:, th * 512: th * 512 + nt]),
                            start=(c == 0), stop=(c == n_in - 1))
                    nc.scalar.activation(
                        dsts[m][:, th * 512: th * 512 + nt], ps[:, :nt],
                        func, bias=bias_col[:, m: m + 1], scale=1.0)

        def vproj(vtiles, srcs, w_name, bias_row, wpool=None, pp=None):
            """Row-major V projection with interleaved ones columns.

            vtiles[t]: [P, H*65]; cols h*65..h*65+63 = V features of head h,
            col h*65+64 = 1.0 (softmax-denominator trick)."""
            w_sb = []
            for c in range(DC):
                w = wpool.tile([P, D], MF, tag="w")
                nc.sync.dma_start(w[:], a[w_name][ts(c, P), :])
                w_sb.append(w)
            for t in range(len(vtiles)):
                ps = pp.tile([P, D], F32, tag="vproj")
                for c in range(DC):
                    nc.tensor.matmul(ps[:], lhsT=_r(srcs[c][:, ts(t, P)]),
                                     rhs=_r(w_sb[c][:]),
                                     start=(c == 0), stop=False)
                nc.tensor.matmul(ps[:], lhsT=_r(ones_row[:]), rhs=_r(bias_row[:]),
                                 start=False, stop=True)
                vt = vtiles[t]
                v3 = vt[:].rearrange("p (h e) -> p h e", e=65)
                nc.scalar.copy(v3[:, :, 0:64],
                               ps[:].rearrange("p (h e) -> p h e", e=64))
                nc.scalar.add(v3[:, :, 64:65],
                              zscr[:, 0:8].rearrange("p (h e) -> p h e", e=1), 1.0)

        def attention(kT, vtiles, qT, bias_col, use_masks, mergedT, pools):
            sp, avp, rp, sbp = pools
            for h in range(H):
                hc, off = h // 2, (h % 2) * DKH
                av = avp.tile([DKH + 1, TOWN], F32, tag="av")
                for kc in range(8):
                    s = sp.tile([P, TOWN], F32, tag="s", bufs=4)
                    masked = use_masks and kc >= 4
                    nc.tensor.matmul(
                        s[:], lhsT=_r(kT[hc][off:off + DKH, ts(kc, P)]),
                        rhs=_r(qT[hc][off:off + DKH, :]), start=True,
                        stop=not masked)
                    if masked:
                        nc.tensor.matmul(s[:], lhsT=ident_r[:],
                                         rhs=masks[:, kc - 4, :],
                                         start=False, stop=True)
                    pt = sbp.tile([P, TOWN], MF, tag="pT", bufs=4)
                    nc.scalar.activation(pt[:], s[:], AF.Exp,
                                         bias=bias_col[:, kc: kc + 1], scale=0.125)
                    nc.tensor.matmul(av[:],
                                     lhsT=_r(vtiles[kc][:, h * 65: h * 65 + 65]),
                                     rhs=_r(pt[:]), start=(kc == 0), stop=(kc == 7))
                srow = sbp.tile([1, TOWN], MF, tag="srow", bufs=2)
                nc.scalar.copy(srow[:], av[DKH: DKH + 1, :])
                R = rp.tile([DKH, TOWN], F32, tag="R")
                nc.tensor.matmul(R[:], lhsT=_r(ones_row[:, 0:DKH]), rhs=_r(srow[:]),
                                 start=True, stop=True)
                rinv = sbp.tile([DKH, TOWN], F32, tag="rinv", bufs=2)
                nc.vector.reciprocal(rinv[:], R[:])
                nc.vector.tensor_tensor(mergedT[hc][off:off + DKH, :],
                                        av[0:DKH, :], rinv[:], op=OP.mult)

        def layernorm(srcs, i, dsts, pools):
            """dsts = LN(srcs) with gain/bias pack i (feature-major chunks)."""
            statp, bcp, sbp = pools
            ssum = statp.tile([1, TOWN], F32, tag="ssum")
            ssq = statp.tile([1, TOWN], F32, tag="ssq")
            for c in range(DC):
                nc.tensor.matmul(ssum[:], lhsT=_r(ones_col[:]), rhs=_r(srcs[c][:]),
                                 start=(c == 0), stop=(c == DC - 1))
            for c in range(DC):
                sq = sbp.tile([P, TOWN], MF, tag="sq", bufs=2)
                nc.scalar.square(sq[:], srcs[c][:])
                nc.tensor.matmul(ssq[:], lhsT=_r(ones_col[:]), rhs=_r(sq[:]),
                                 start=(c == 0), stop=(c == DC - 1))
            mu = sbp.tile([1, TOWN], MF, tag="mu", bufs=1)
            nc.scalar.mul(mu[:], ssum[:], 1.0 / D)
            ex2 = sbp.tile([1, TOWN], F32, tag="ex2", bufs=1)
            nc.scalar.mul(ex2[:], ssq[:], 1.0 / D)
            musq = sbp.tile([1, TOWN], F32, tag="musq", bufs=1)
            nc.scalar.square(musq[:], mu[:])
            var = sbp.tile([1, TOWN], F32, tag="var", bufs=1)
            nc.vector.scalar_tensor_tensor(var[:], in0=musq[:], scalar=-1.0,
                                           in1=ex2[:], op0=OP.mult, op1=OP.add)
            std = sbp.tile([1, TOWN], MF, tag="std", bufs=1)
            nc.scalar.activation(std[:], var[:], AF.Sqrt, bias=eps_c[:], scale=1.0)
            mu_b = bcp.tile([P, TOWN], F32, tag="mu_b", bufs=1)
            nc.tensor.matmul(mu_b[:], lhsT=_r(ones_row[:]), rhs=_r(mu[:]),
                             start=True, stop=True)
            std_b = bcp.tile([P, TOWN], F32, tag="std_b", bufs=1)
            nc.tensor.matmul(std_b[:], lhsT=_r(ones_row[:]), rhs=_r(std[:]),
                             start=True, stop=True)
            ainv = sbp.tile([P, TOWN], F32, tag="ainv", bufs=1)
            nc.vector.reciprocal(ainv[:], std_b[:])
            for c in range(DC):
                t1 = sbp.tile([P, TOWN], F32, tag="lnt", bufs=2)
                nc.vector.tensor_tensor(t1[:], srcs[c][:], mu_b[:], op=OP.subtract)
                t2 = sbp.tile([P, TOWN], F32, tag="lnt2", bufs=2)
                nc.vector.tensor_tensor(t2[:], t1[:], ainv[:], op=OP.mult)
                nc.vector.tensor_scalar(
                    dsts[c][:], t2[:], gc[i][:, c: c + 1], bc[i][:, c: c + 1],
                    op0=OP.mult, op1=OP.add)

        # ================= block A: embed, self-attention, LN1 =================
        with tc.tile_pool(name="blkA", bufs=1) as bA, \
             tc.tile_pool(name="rotA", bufs=3) as rA:
            x0T = [bA.tile([P, T], MF, tag=f"x0T{c}", name=f"x0T{c}") for c in range(DC)]
            kT = [bA.tile([P, T], MF, tag=f"kT{c}", name=f"kT{c}") for c in range(DC)]
            vsb = [bA.tile([P, H * 65], MF, tag=f"v{t}", name=f"v{t}") for t in range(8)]
            qT = [bA.tile([P, TOWN], MF, tag=f"qT{c}", name=f"qT{c}") for c in range(DC)]
            mergedT = [bA.tile([P, TOWN], MF, tag=f"mgT{c}", name=f"mgT{c}") for c in range(DC)]

            # --- embedding gather + transpose + scale + positional encoding ---
            with tc.tile_pool(name="pe", bufs=1) as pep, \
                 tc.tile_pool(name="psA0", bufs=3, space="PSUM") as pp0:
                idx_sb = pep.tile([P, 8], I32, tag="idx")
                nc.sync.dma_start(
                    idx_sb[:], a["idx"].rearrange("(c p) o -> p (c o)", p=P))
                peT_sb = [pep.tile([P, T], F32, tag=f"pe{c}", name=f"pe{c}") for c in range(DC)]
                for c in range(DC):
                    nc.sync.dma_start(peT_sb[c][:], a["peT"][c, :, :])
                for t in range(8):
                    xg = rA.tile([P, D], F32, tag="xg")
                    nc.gpsimd.indirect_dma_start(
                        out=xg[:], out_offset=None, in_=a["emb"][:, :],
                        in_offset=bass.IndirectOffsetOnAxis(
                            ap=idx_sb[:, t: t + 1], axis=0))
                    for c in range(DC):
                        tp = pp0.tile([P, P], F32, tag="tp")
                        nc.tensor.transpose(tp[:], xg[:, ts(c, P)], ident[:])
                        nc.vector.scalar_tensor_tensor(
                            x0T[c][:, ts(t, P)], in0=tp[:], scalar=SQRT_D,
                            in1=peT_sb[c][:, ts(t, P)], op0=OP.mult, op1=OP.add)

            # --- K, V, Q projections ---
            with tc.tile_pool(name="wA", bufs=8) as wp, \
                 tc.tile_pool(name="psA1", bufs=3, space="PSUM") as pp1:
                proj_fm(kT, x0T, "WkT", bk_c, T, wpool=wp, pp=pp1)
                vproj(vsb, x0T, "WvT", bv_r, wpool=wp, pp=pp1)
                proj_fm(qT, [x0T[c][:, 512:1024] for c in range(DC)],
                        "WqT", bq_c, TOWN, wpool=wp, pp=pp1)

            # --- causal self-attention ---
            with tc.tile_pool(name="psS", bufs=3, space="PSUM") as sp, \
                 tc.tile_pool(name="psAV", bufs=2, space="PSUM") as avp, \
                 tc.tile_pool(name="psR", bufs=2, space="PSUM") as rp, \
                 tc.tile_pool(name="sbA", bufs=3) as sbp:
                attention(kT, vsb, qT, biasS, True, mergedT, (sp, avp, rp, sbp))

            # --- Wo1 + residual + LN1 -> x1T ---
            with tc.tile_pool(name="wA2", bufs=4) as wp, \
                 tc.tile_pool(name="psA2", bufs=2, space="PSUM") as pp2, \
                 tc.tile_pool(name="psStat", bufs=1, space="PSUM") as statp, \
                 tc.tile_pool(name="psBC", bufs=2, space="PSUM") as bcp, \
                 tc.tile_pool(name="sbLN", bufs=3) as sbp:
                w_sb = []
                for c in range(DC):
                    w = wp.tile([P, D], MF, tag="w")
                    nc.sync.dma_start(w[:], a["Wo1T"][ts(c, P), :])
                    w_sb.append(w)
                ln_in = []
                for m in range(DC):
                    ps = pp2.tile([P, TOWN], F32, tag="proj")
                    for c in range(DC):
                        nc.tensor.matmul(ps[:], lhsT=_r(w_sb[c][:, ts(m, P)]),
                                         rhs=_r(mergedT[c][:]),
                                         start=(c == 0), stop=(c == DC - 1))
                    li = sbp.tile([P, TOWN], MF, tag=f"li{m}", name=f"li{m}", bufs=1)
                    nc.vector.scalar_tensor_tensor(
                        li[:], in0=ps[:], scalar=bo1_c[:, m: m + 1],
                        in1=x0T[m][:, 512:1024], op0=OP.add, op1=OP.add)
                    ln_in.append(li)
                layernorm(ln_in, 1, x1T, (statp, bcp, sbp))

        # ================= block B: cross-attention, LN2 =================
        with tc.tile_pool(name="blkB", bufs=1) as bB, \
             tc.tile_pool(name="rotB", bufs=3) as rB:
            encT = [bB.tile([P, T], MF, tag=f"encT{c}", name=f"encT{c}") for c in range(DC)]
            ekT = [bB.tile([P, T], MF, tag=f"ekT{c}", name=f"ekT{c}") for c in range(DC)]
            evsb = [bB.tile([P, H * 65], MF, tag=f"ev{t}", name=f"ev{t}") for t in range(8)]
            cqT = [bB.tile([P, TOWN], MF, tag=f"cqT{c}", name=f"cqT{c}") for c in range(DC)]
            mergedT2 = [bB.tile([P, TOWN], MF, tag=f"mg2T{c}", name=f"mg2T{c}") for c in range(DC)]

            with tc.tile_pool(name="psB0", bufs=3, space="PSUM") as pp0:
                for t in range(8):
                    es = rB.tile([P, D], F32, tag="es")
                    nc.sync.dma_start(es[:], a["enc"][ts(t, P), :])
                    for c in range(DC):
                        tp = pp0.tile([P, P], F32, tag="tp")
                        nc.tensor.transpose(tp[:], es[:, ts(c, P)], ident[:])
                        nc.scalar.copy(encT[c][:, ts(t, P)], tp[:])

            with tc.tile_pool(name="wB", bufs=8) as wp, \
                 tc.tile_pool(name="psB1", bufs=3, space="PSUM") as pp1:
                proj_fm(ekT, encT, "eWkT", ebk_c, T, wpool=wp, pp=pp1)
                vproj(evsb, encT, "eWvT", ebv_r, wpool=wp, pp=pp1)
                proj_fm(cqT, x1T, "cWqT", cbq_c, TOWN, wpool=wp, pp=pp1)

            with tc.tile_pool(name="psS", bufs=3, space="PSUM") as sp, \
                 tc.tile_pool(name="psAV", bufs=2, space="PSUM") as avp, \
                 tc.tile_pool(name="psR", bufs=2, space="PSUM") as rp, \
                 tc.tile_pool(name="sbB", bufs=3) as sbp:
                attention(ekT, evsb, cqT, biasC, False, mergedT2,
                          (sp, avp, rp, sbp))

            with tc.tile_pool(name="wB2", bufs=4) as wp, \
                 tc.tile_pool(name="psB2", bufs=2, space="PSUM") as pp2, \
                 tc.tile_pool(name="psStat", bufs=1, space="PSUM") as statp, \
                 tc.tile_pool(name="psBC", bufs=2, space="PSUM") as bcp, \
                 tc.tile_pool(name="sbLN", bufs=3) as sbp:
                w_sb = []
                for c in range(DC):
                    w = wp.tile([P, D], MF, tag="w")
                    nc.sync.dma_start(w[:], a["Wo2T"][ts(c, P), :])
                    w_sb.append(w)
                ln_in = []
                for m in range(DC):
                    ps = pp2.tile([P, TOWN], F32, tag="proj")
                    for c in range(DC):
                        nc.tensor.matmul(ps[:], lhsT=_r(w_sb[c][:, ts(m, P)]),
                                         rhs=_r(mergedT2[c][:]),
                                         start=(c == 0), stop=(c == DC - 1))
                    li = sbp.tile([P, TOWN], MF, tag=f"li{m}", name=f"li{m}", bufs=1)
                    nc.vector.scalar_tensor_tensor(
                        li[:], in0=ps[:], scalar=bo2_c[:, m: m + 1],
                        in1=x1T[m][:], op0=OP.add, op1=OP.add)
                    ln_in.append(li)
                layernorm(ln_in, 2, x2T, (statp, bcp, sbp))

        # ================= block C: FFN, LN3 =================
        with tc.tile_pool(name="wC", bufs=4) as wp1, \
             tc.tile_pool(name="wC2", bufs=8) as wp2, \
             tc.tile_pool(name="hC", bufs=1) as hp, \
             tc.tile_pool(name="psC", bufs=3, space="PSUM") as pp, \
             tc.tile_pool(name="psStat", bufs=1, space="PSUM") as statp, \
             tc.tile_pool(name="psBC", bufs=2, space="PSUM") as bcp, \
             tc.tile_pool(name="sbC", bufs=3) as sbp:
            hT = [hp.tile([P, TOWN], MF, tag=f"hT{m}", name=f"hT{m}") for m in range(FFN // P)]
            proj_fm(hT, x2T, "W1T", b1_c, TOWN, func=AF.Relu, wpool=wp1, pp=pp)
            w_sb = []
            for c in range(FFN // P):
                w = wp2.tile([P, D], MF, tag="w2")
                nc.sync.dma_start(w[:], a["W2T"][ts(c, P), :])
                w_sb.append(w)
            ln_in = []
            for m in range(DC):
                ps = pp.tile([P, TOWN], F32, tag="proj")
                for c in range(FFN // P):
                    nc.tensor.matmul(ps[:], lhsT=_r(w_sb[c][:, ts(m, P)]),
                                     rhs=_r(hT[c][:]),
                                     start=(c == 0), stop=(c == FFN // P - 1))
                li = sbp.tile([P, TOWN], MF, tag=f"li{m}", name=f"li{m}", bufs=1)
                nc.vector.scalar_tensor_tensor(
                    li[:], in0=ps[:], scalar=b2_c[:, m: m + 1], in1=x2T[m][:],
                    op0=OP.add, op1=OP.add)
                ln_in.append(li)
            layernorm(ln_in, 3, x3T, (statp, bcp, sbp))

        # ================= block D: vocab projection =================
        with tc.tile_pool(name="wD", bufs=12) as wp, \
             tc.tile_pool(name="x3B", bufs=1) as xbp, \
             tc.tile_pool(name="stD", bufs=6) as stp, \
             tc.tile_pool(name="psD", bufs=2, space="PSUM") as pp:
            if WOUT_DT is BF16:
                x3v = [xbp.tile([P, TOWN], BF16, tag=f"x3B{c}", name=f"x3B{c}")
                       for c in range(DC)]
                for c in range(DC):
                    nc.scalar.copy(x3v[c][:], x3T[c][:])
            else:
                x3v = x3T
            for vg in range(NVG):
                w_sb = []
                for c in range(DC):
                    w = wp.tile([P, VG * VCH], WOUT_DT, tag="wo")
                    nc.sync.dma_start(
                        w[:], a["WoutT"][ts(c, P),
                                         vg * VG * VCH:(vg + 1) * VG * VCH])
                    w_sb.append(w)
                for t in range(TOWN // P):
                    ps = pp.tile([P, VG, 512], F32, tag="vps")
                    for j in range(VG):
                        for c in range(DC):
                            nc.tensor.matmul(
                                ps[:, j, 0:VCH],
                                lhsT=x3v[c][:, ts(t, P)],
                                rhs=w_sb[c][:, ts(j, VCH)],
                                start=(c == 0), stop=(c == DC - 1))
                    stage = stp.tile([P, VG * VCH], F32, tag="stage")
                    st3 = stage[:].rearrange("p (j e) -> p j e", e=VCH)
                    if t % 2 == 0:
                        nc.scalar.copy(st3, ps[:, :, 0:VCH])
                    else:
                        nc.vector.tensor_copy(st3, ps[:, :, 0:VCH])
                    nc.sync.dma_start(
                        a["out"][ts(t, P), vg * VG * VCH:(vg + 1) * VG * VCH],
                        stage[:])


# --------------------------------------------------------------------------
# host-side input preparation
# --------------------------------------------------------------------------

def _pos_encoding_np(t, d):
    pos = np.arange(t, dtype=np.float32)[:, None]
    freqs = 1.0 / (10000.0 ** (np.arange(0, d, 2, dtype=np.float32) / d))
    pe = np.zeros((t, d), np.float32)
    pe[:, 0::2] = np.sin(pos * freqs)
    pe[:, 1::2] = np.cos(pos * freqs)
    return pe


def _col_pack(b):
    """[n] -> [P, n//P] with element (p, c) = b[c*P + p]."""
    b = np.asarray(b, np.float32)
    return np.ascontiguousarray(b.reshape(-1, P).T)


def prep_in_maps(inputs):
    gi = lambda n: np.asarray(inputs[n])
    tokens = gi("tokens").astype(np.int32)                      # [4, 1024]
    enc_all = np.ascontiguousarray(gi("enc_embeddings").astype(np.float32))
    enc_pad = gi("enc_pad_mask").astype(bool)
    emb = np.ascontiguousarray(gi("emb").astype(np.float32))

    shared = {"emb": emb}
    for nm in ("Wq", "Wk", "Wv", "Wo1", "cWq", "eWk", "eWv", "Wo2", "W1", "W2",
               "Wout"):
        shared[nm + "T"] = np.ascontiguousarray(
            gi(nm).astype(np.float32).T)
    if WOUT_DT is BF16:
        import ml_dtypes
        shared["WoutT"] = shared["WoutT"].astype(ml_dtypes.bfloat16)
    for nm, src in (("bq", "bq"), ("bk", "bk"), ("bo1", "bo1"), ("cbq", "cbq"),
                    ("ebk", "ebk"), ("bo2", "bo2"), ("b2", "b2"), ("b1", "b1")):
        shared[nm + "_c"] = _col_pack(gi(src))
    shared["bv_r"] = gi("bv").astype(np.float32).reshape(1, D)
    shared["ebv_r"] = gi("ebv").astype(np.float32).reshape(1, D)
    for i, (g, b) in ((1, ("g1", "be1")), (2, ("g3", "be3")), (3, ("g2", "be2"))):
        shared[f"gc{i}"] = _col_pack(gi(g))
        shared[f"bc{i}"] = _col_pack(gi(b))

    # causal diagonal-block masks for key chunks 4..7 (slot space)
    kk = np.arange(P)[:, None]
    qq = np.arange(TOWN)[None, :]
    masks = np.zeros((P, 4, TOWN), np.float32)
    for j in range(4):
        masks[:, j, :] = np.where((j * P + kk) > qq, NEG, 0.0)
    shared["masks"] = masks

    pe = _pos_encoding_np(T, D)

    in_maps = []
    for core in range(8):
        b, hf = core // 2, core % 2
        own = tokens[b, hf * 512:(hf + 1) * 512]
        idx_full = np.concatenate([tokens[b, :512], own])        # [1024]
        pe_slots = np.concatenate([pe[:512], pe[hf * 512:(hf + 1) * 512]], axis=0)
        peT = np.ascontiguousarray(
            pe_slots.T.reshape(DC, P, T, order="C"))             # careful below
        # pe_slots.T is [D, T]; reshape to [DC, P, T] splits D into chunks
        biasS = np.where(idx_full == PAD_ID, NEG, 0.0).astype(np.float32)
        if hf == 0:
            biasS[:512] = NEG                                    # no prefix half
        biasC = np.where(enc_pad[b], NEG, 0.0).astype(np.float32)
        m = dict(shared)
        m["idx"] = np.ascontiguousarray(idx_full.reshape(T, 1))
        m["peT"] = peT
        m["enc"] = np.ascontiguousarray(enc_all[b])
        m["biasS"] = np.ascontiguousarray(biasS.reshape(8, P).T)
        m["biasC"] = np.ascontiguousarray(biasC.reshape(8, P).T)
        in_maps.append(m)
    return in_maps


def assemble(results, inputs):
    full = np.empty((4, 1024, V), np.float32)
    for core in range(8):
        b, hf = core // 2, core % 2
        full[b, hf * 512:(hf + 1) * 512] = results[core]["out"]
    bout = np.asarray(inputs["bout"], np.float32)
    if np.any(bout):
        full += bout[None, None, :]
    return full


# --------------------------------------------------------------------------
# public entry point
# --------------------------------------------------------------------------

def kernel(**inputs):
    from concourse.bass_utils import run_bass_kernel_spmd
    nc = build_module()
    in_maps = prep_in_maps(inputs)
    res = run_bass_kernel_spmd(nc, in_maps, core_ids=list(range(8)))
    return assemble(res.results, inputs)


if __name__ == "__main__":
    nc = build_module()
    print("built ok")


# revision 17
# speedup vs baseline: 1.3523x; 1.1319x over previous
"""Trainium2 Bass kernel for nn_DecoderTrans (dense transformer decoder layer + vocab head).

Sharding: 8 cores = (batch b, half hf). Each core computes the full trunk for its
512 "own" tokens (queries) and the K/V context for the whole 1024-token sequence
of its batch element. Own tokens always occupy key slots [512, 1024) so the
program is uniform SPMD; per-core mask/bias DATA encodes the causal structure.
Activations are kept feature-major (x^T: [D, tokens]) throughout; weights are
shipped pre-transposed ([d_in, d_out]).
"""
import math
import os
import sys

sys.path.insert(0, "/opt/trn_rl_repo")

import numpy as np

import concourse.bass as bass
import concourse.tile as tile
from concourse import bacc, mybir
from concourse.bass import ts
from concourse.masks import make_identity

P = 128
D = 512
DC = D // P          # 4 feature chunks
T = 1024             # full sequence (keys)
TOWN = 512           # own tokens per core (queries), slots [512, 1024)
H = 8
DKH = 64             # head dim
V = 32000
VCH = 500            # vocab columns per matmul (fits PSUM bank, >=256 for f32r)
VG = 4               # vocab chunks per group
NVG = V // (VCH * VG)  # 16 groups
FFN = 2 * D
NEG = -30000.0
SQRT_D = math.sqrt(D)
PAD_ID = 0

F32 = mybir.dt.float32
F32R = mybir.dt.float32r
I32 = mybir.dt.int32
AF = mybir.ActivationFunctionType
OP = mybir.AluOpType

# matmul input dtype: float32r streams 4x faster than float32 on TRN2 PE.
# fp32r is a rounded format: every producer writing a matmul operand must
# declare its output float32r, so operand tiles/DRAM tensors use MF dtype.
MM_DT = F32R if os.environ.get("KMM", "r") == "r" else F32
MF = MM_DT
BF16 = mybir.dt.bfloat16
# vocab projection operand dtype: bf16 halves the Wout DMA stream (the vocab
# stage is HBM-bound); 'r' keeps fp32r end to end
WOUT_DT = BF16 if os.environ.get("KWOUT", "b") == "b" else MM_DT
# output staging dtype: bf16 halves the 64MB/core logit writeback (HBM-bound)
OUT_DT = BF16 if os.environ.get("KOUT", "b") == "b" else F32


def _r(ap):
    return ap


# --------------------------------------------------------------------------
# program builder
# --------------------------------------------------------------------------

def build_module():
    nc = bacc.Bacc("TRN2", target_bir_lowering=False, debug=False)

    def din(name, shape, dt=F32):
        return nc.dram_tensor(name, shape, dt, kind="ExternalInput").ap()

    a = {}
    a["idx"] = din("idx", [T, 1], I32)
    a["emb"] = din("emb", [V, D])
    a["peT"] = din("peT", [DC, P, T])
    a["enc"] = din("enc", [T, D])
    a["masks"] = din("masks", [P, 4, TOWN], MF)
    a["biasS"] = din("biasS", [P, 8])
    a["biasC"] = din("biasC", [P, 8])
    for nm in ("WqT", "WkT", "WvT", "Wo1T", "cWqT", "eWkT", "eWvT", "Wo2T"):
        a[nm] = din(nm, [D, D], MF)
    a["W1T"] = din("W1T", [D, FFN], MF)
    a["W2T"] = din("W2T", [FFN, D], MF)
    a["WoutT"] = din("WoutT", [D, V], WOUT_DT)
    # per-partition bias columns [P, n_out_chunks]
    for nm in ("bq", "bk", "bo1", "cbq", "ebk", "bo2", "b2"):
        a[nm + "_c"] = din(nm + "_c", [P, DC])
    a["b1_c"] = din("b1_c", [P, FFN // P])
    # bias rows for row-major (V) projections
    a["bv_r"] = din("bv_r", [1, D], MF)
    a["ebv_r"] = din("ebv_r", [1, D], MF)
    # layernorm gain/bias per-partition packs [P, DC]
    for i in (1, 2, 3):
        a[f"gc{i}"] = din(f"gc{i}", [P, DC])
        a[f"bc{i}"] = din(f"bc{i}", [P, DC])
    out = nc.dram_tensor("out", [TOWN, V], OUT_DT, kind="ExternalOutput").ap()
    a["out"] = out

    with tile.TileContext(nc) as tc, \
         nc.allow_low_precision(reason="fp32r matmul operand pipeline"):
        _emit(tc, a)
    nc.compile()
    return nc


def _emit(tc, a):
    nc = tc.nc

    with tc.tile_pool(name="const", bufs=1) as cp, \
         tc.tile_pool(name="trunk", bufs=1) as trunkp:
        # ---- constants ----
        ident = cp.tile([P, P], F32, tag="ident")
        make_identity(nc, ident[:])
        ident_r = cp.tile([P, P], MF, tag="ident_r")
        nc.scalar.copy(ident_r[:], ident[:])
        zscr = cp.tile([P, TOWN], F32, tag="zscr")
        nc.vector.memset(zscr[:], 0.0)
        ones_col = cp.tile([P, 1], MF, tag="ones_col")
        nc.scalar.add(ones_col[:], zscr[:, 0:1], 1.0)
        ones_row = cp.tile([1, P], MF, tag="ones_row")
        nc.scalar.add(ones_row[:], zscr[0:1, 0:P], 1.0)
        eps_c = cp.tile([1, 1], F32, tag="eps_c")
        nc.vector.memset(eps_c[:], 1e-5)
        biasS = cp.tile([P, 8], F32, tag="biasS")
        nc.sync.dma_start(biasS[:], a["biasS"][:, :])
        biasC = cp.tile([P, 8], F32, tag="biasC")
        nc.sync.dma_start(biasC[:], a["biasC"][:, :])
        masks = cp.tile([P, 4, TOWN], MF, tag="masks")
        nc.sync.dma_start(masks[:], a["masks"][:, :, :])

        def load_bias_col(nm, nch):
            t = cp.tile([P, nch], F32, tag=nm)
            nc.sync.dma_start(t[:], a[nm][:, :])
            return t
        bq_c = load_bias_col("bq_c", DC)
        bk_c = load_bias_col("bk_c", DC)
        bo1_c = load_bias_col("bo1_c", DC)
        cbq_c = load_bias_col("cbq_c", DC)
        ebk_c = load_bias_col("ebk_c", DC)
        bo2_c = load_bias_col("bo2_c", DC)
        b2_c = load_bias_col("b2_c", DC)
        b1_c = load_bias_col("b1_c", FFN // P)

        def load_row(nm, n):
            t = cp.tile([1, n], MF, tag=nm)
            nc.sync.dma_start(t[:], a[nm][:, :])
            return t
        bv_r = load_row("bv_r", D)
        ebv_r = load_row("ebv_r", D)
        gc = {i: None for i in (1, 2, 3)}
        bc = {i: None for i in (1, 2, 3)}
        for i in (1, 2, 3):
            gc[i] = load_bias_col(f"gc{i}", DC)
            bc[i] = load_bias_col(f"bc{i}", DC)

        # ---- long-lived trunk activations ----
        x1T = [trunkp.tile([P, TOWN], MF, tag=f"x1T{c}", name=f"x1T{c}") for c in range(DC)]
        x2T = [trunkp.tile([P, TOWN], MF, tag=f"x2T{c}", name=f"x2T{c}") for c in range(DC)]
        x3T = [trunkp.tile([P, TOWN], MF, tag=f"x3T{c}", name=f"x3T{c}") for c in range(DC)]

        # ================= shared helpers =================

        def proj_fm(dsts, srcs, w_name, bias_col, ntok, func=AF.Identity,
                    wpool=None, pp=None, n_in=DC):
            """dsts[m][:, :ntok] = func(W @ srcs + b); feature-major in/out."""
            w_sb = []
            for c in range(n_in):
                w = wpool.tile([P, len(dsts) * P], MF, tag="w")
                nc.sync.dma_start(w[:], a[w_name][ts(c, P), :])
                w_sb.append(w)
            nth = (ntok + 511) // 512
            for m in range(len(dsts)):
                for th in range(nth):
                    nt = min(512, ntok - th * 512)
                    ps = pp.tile([P, 512], F32, tag="proj")
                    for c in range(n_in):
                        nc.tensor.matmul(
                            ps[:, :nt],
                            lhsT=_r(w_sb[c][:, ts(m, P)]),
                            rhs=_r(srcs[c][:, th * 512: th * 512 + nt]),
                            start=(c == 0), stop=(c == n_in - 1))
                    nc.scalar.activation(
                        dsts[m][:, th * 512: th * 512 + nt], ps[:, :nt],
                        func, bias=bias_col[:, m: m + 1], scale=1.0)

        def vproj(vtiles, srcs, w_name, bias_row, wpool=None, pp=None):
            """Row-major V projection with interleaved ones columns.

            vtiles[t]: [P, H*65]; cols h*65..h*65+63 = V features of head h,
            col h*65+64 = 1.0 (softmax-denominator trick)."""
            w_sb = []
            for c in range(DC):
                w = wpool.tile([P, D], MF, tag="w")
                nc.sync.dma_start(w[:], a[w_name][ts(c, P), :])
                w_sb.append(w)
            for t in range(len(vtiles)):
                ps = pp.tile([P, D], F32, tag="vproj")
                for c in range(DC):
                    nc.tensor.matmul(ps[:], lhsT=_r(srcs[c][:, ts(t, P)]),
                                     rhs=_r(w_sb[c][:]),
                                     start=(c == 0), stop=False)
                nc.tensor.matmul(ps[:], lhsT=_r(ones_row[:]), rhs=_r(bias_row[:]),
                                 start=False, stop=True)
                vt = vtiles[t]
                v3 = vt[:].rearrange("p (h e) -> p h e", e=65)
                nc.scalar.copy(v3[:, :, 0:64],
                               ps[:].rearrange("p (h e) -> p h e", e=64))
                nc.scalar.add(v3[:, :, 64:65],
                              zscr[:, 0:8].rearrange("p (h e) -> p h e", e=1), 1.0)

        def attention(kT, vtiles, qT, bias_col, use_masks, mergedT, pools):
            sp, avp, rp, sbp = pools
            for h in range(H):
                hc, off = h // 2, (h % 2) * DKH
                av = avp.tile([DKH + 1, TOWN], F32, tag="av")
                for kc in range(8):
                    s = sp.tile([P, TOWN], F32, tag="s", bufs=4)
                    masked = use_masks and kc >= 4
                    nc.tensor.matmul(
                        s[:], lhsT=_r(kT[hc][off:off + DKH, ts(kc, P)]),
                        rhs=_r(qT[hc][off:off + DKH, :]), start=True,
                        stop=not masked)
                    if masked:
                        nc.tensor.matmul(s[:], lhsT=ident_r[:],
                                         rhs=masks[:, kc - 4, :],
                                         start=False, stop=True)
                    pt = sbp.tile([P, TOWN], MF, tag="pT", bufs=4)
                    nc.scalar.activation(pt[:], s[:], AF.Exp,
                                         bias=bias_col[:, kc: kc + 1], scale=0.125)
                    nc.tensor.matmul(av[:],
                                     lhsT=_r(vtiles[kc][:, h * 65: h * 65 + 65]),
                                     rhs=_r(pt[:]), start=(kc == 0), stop=(kc == 7))
                srow = sbp.tile([1, TOWN], MF, tag="srow", bufs=2)
                nc.scalar.copy(srow[:], av[DKH: DKH + 1, :])
                R = rp.tile([DKH, TOWN], F32, tag="R")
                nc.tensor.matmul(R[:], lhsT=_r(ones_row[:, 0:DKH]), rhs=_r(srow[:]),
                                 start=True, stop=True)
                rinv = sbp.tile([DKH, TOWN], F32, tag="rinv", bufs=2)
                nc.vector.reciprocal(rinv[:], R[:])
                nc.vector.tensor_tensor(mergedT[hc][off:off + DKH, :],
                                        av[0:DKH, :], rinv[:], op=OP.mult)

        def layernorm(srcs, i, dsts, pools):
            """dsts = LN(srcs) with gain/bias pack i (feature-major chunks)."""
            statp, bcp, sbp = pools
            ssum = statp.tile([1, TOWN], F32, tag="ssum")
            ssq = statp.tile([1, TOWN], F32, tag="ssq")
            for c in range(DC):
                nc.tensor.matmul(ssum[:], lhsT=_r(ones_col[:]), rhs=_r(srcs[c][:]),
                                 start=(c == 0), stop=(c == DC - 1))
            for c in range(DC):
                sq = sbp.tile([P, TOWN], MF, tag="sq", bufs=2)
                nc.scalar.square(sq[:], srcs[c][:])
                nc.tensor.matmul(ssq[:], lhsT=_r(ones_col[:]), rhs=_r(sq[:]),
                                 start=(c == 0), stop=(c == DC - 1))
            mu = sbp.tile([1, TOWN], MF, tag="mu", bufs=1)
            nc.scalar.mul(mu[:], ssum[:], 1.0 / D)
            ex2 = sbp.tile([1, TOWN], F32, tag="ex2", bufs=1)
            nc.scalar.mul(ex2[:], ssq[:], 1.0 / D)
            musq = sbp.tile([1, TOWN], F32, tag="musq", bufs=1)
            nc.scalar.square(musq[:], mu[:])
            var = sbp.tile([1, TOWN], F32, tag="var", bufs=1)
            nc.vector.scalar_tensor_tensor(var[:], in0=musq[:], scalar=-1.0,
                                           in1=ex2[:], op0=OP.mult, op1=OP.add)
            std = sbp.tile([1, TOWN], MF, tag="std", bufs=1)
            nc.scalar.activation(std[:], var[:], AF.Sqrt, bias=eps_c[:], scale=1.0)
            mu_b = bcp.tile([P, TOWN], F32, tag="mu_b", bufs=1)
            nc.tensor.matmul(mu_b[:], lhsT=_r(ones_row[:]), rhs=_r(mu[:]),
                             start=True, stop=True)
            std_b = bcp.tile([P, TOWN], F32, tag="std_b", bufs=1)
            nc.tensor.matmul(std_b[:], lhsT=_r(ones_row[:]), rhs=_r(std[:]),
                             start=True, stop=True)
            ainv = sbp.tile([P, TOWN], F32, tag="ainv", bufs=1)
            nc.vector.reciprocal(ainv[:], std_b[:])
            for c in range(DC):
                t1 = sbp.tile([P, TOWN], F32, tag="lnt", bufs=2)
                nc.vector.tensor_tensor(t1[:], srcs[c][:], mu_b[:], op=OP.subtract)
                t2 = sbp.tile([P, TOWN], F32, tag="lnt2", bufs=2)
                nc.vector.tensor_tensor(t2[:], t1[:], ainv[:], op=OP.mult)
                nc.vector.tensor_scalar(
                    dsts[c][:], t2[:], gc[i][:, c: c + 1], bc[i][:, c: c + 1],
                    op0=OP.mult, op1=OP.add)

        # ================= block A: embed, self-attention, LN1 =================
        with tc.tile_pool(name="blkA", bufs=1) as bA, \
             tc.tile_pool(name="rotA", bufs=3) as rA:
            x0T = [bA.tile([P, T], MF, tag=f"x0T{c}", name=f"x0T{c}") for c in range(DC)]
            kT = [bA.tile([P, T], MF, tag=f"kT{c}", name=f"kT{c}") for c in range(DC)]
            vsb = [bA.tile([P, H * 65], MF, tag=f"v{t}", name=f"v{t}") for t in range(8)]
            qT = [bA.tile([P, TOWN], MF, tag=f"qT{c}", name=f"qT{c}") for c in range(DC)]
            mergedT = [bA.tile([P, TOWN], MF, tag=f"mgT{c}", name=f"mgT{c}") for c in range(DC)]

            # --- embedding gather + transpose + scale + positional encoding ---
            with tc.tile_pool(name="pe", bufs=1) as pep, \
                 tc.tile_pool(name="psA0", bufs=3, space="PSUM") as pp0:
                idx_sb = pep.tile([P, 8], I32, tag="idx")
                nc.sync.dma_start(
                    idx_sb[:], a["idx"].rearrange("(c p) o -> p (c o)", p=P))
                peT_sb = [pep.tile([P, T], F32, tag=f"pe{c}", name=f"pe{c}") for c in range(DC)]
                for c in range(DC):
                    nc.sync.dma_start(peT_sb[c][:], a["peT"][c, :, :])
                for t in range(8):
                    xg = rA.tile([P, D], F32, tag="xg")
                    nc.gpsimd.indirect_dma_start(
                        out=xg[:], out_offset=None, in_=a["emb"][:, :],
                        in_offset=bass.IndirectOffsetOnAxis(
                            ap=idx_sb[:, t: t + 1], axis=0))
                    for c in range(DC):
                        tp = pp0.tile([P, P], F32, tag="tp")
                        nc.tensor.transpose(tp[:], xg[:, ts(c, P)], ident[:])
                        nc.vector.scalar_tensor_tensor(
                            x0T[c][:, ts(t, P)], in0=tp[:], scalar=SQRT_D,
                            in1=peT_sb[c][:, ts(t, P)], op0=OP.mult, op1=OP.add)

            # --- K, V, Q projections ---
            with tc.tile_pool(name="wA", bufs=8) as wp, \
                 tc.tile_pool(name="psA1", bufs=3, space="PSUM") as pp1:
                proj_fm(kT, x0T, "WkT", bk_c, T, wpool=wp, pp=pp1)
                vproj(vsb, x0T, "WvT", bv_r, wpool=wp, pp=pp1)
                proj_fm(qT, [x0T[c][:, 512:1024] for c in range(DC)],
                        "WqT", bq_c, TOWN, wpool=wp, pp=pp1)

            # --- causal self-attention ---
            with tc.tile_pool(name="psS", bufs=3, space="PSUM") as sp, \
                 tc.tile_pool(name="psAV", bufs=2, space="PSUM") as avp, \
                 tc.tile_pool(name="psR", bufs=2, space="PSUM") as rp, \
                 tc.tile_pool(name="sbA", bufs=3) as sbp:
                attention(kT, vsb, qT, biasS, True, mergedT, (sp, avp, rp, sbp))

            # --- Wo1 + residual + LN1 -> x1T ---
            with tc.tile_pool(name="wA2", bufs=4) as wp, \
                 tc.tile_pool(name="psA2", bufs=2, space="PSUM") as pp2, \
                 tc.tile_pool(name="psStat", bufs=1, space="PSUM") as statp, \
                 tc.tile_pool(name="psBC", bufs=2, space="PSUM") as bcp, \
                 tc.tile_pool(name="sbLN", bufs=3) as sbp:
                w_sb = []
                for c in range(DC):
                    w = wp.tile([P, D], MF, tag="w")
                    nc.sync.dma_start(w[:], a["Wo1T"][ts(c, P), :])
                    w_sb.append(w)
                ln_in = []
                for m in range(DC):
                    ps = pp2.tile([P, TOWN], F32, tag="proj")
                    for c in range(DC):
                        nc.tensor.matmul(ps[:], lhsT=_r(w_sb[c][:, ts(m, P)]),
                                         rhs=_r(mergedT[c][:]),
                                         start=(c == 0), stop=(c == DC - 1))
                    li = sbp.tile([P, TOWN], MF, tag=f"li{m}", name=f"li{m}", bufs=1)
                    nc.vector.scalar_tensor_tensor(
                        li[:], in0=ps[:], scalar=bo1_c[:, m: m + 1],
                        in1=x0T[m][:, 512:1024], op0=OP.add, op1=OP.add)
                    ln_in.append(li)
                layernorm(ln_in, 1, x1T, (statp, bcp, sbp))

        # ================= block B: cross-attention, LN2 =================
        with tc.tile_pool(name="blkB", bufs=1) as bB, \
             tc.tile_pool(name="rotB", bufs=3) as rB:
            encT = [bB.tile([P, T], MF, tag=f"encT{c}", name=f"encT{c}") for c in range(DC)]
            ekT = [bB.tile([P, T], MF, tag=f"ekT{c}", name=f"ekT{c}") for c in range(DC)]
            evsb = [bB.tile([P, H * 65], MF, tag=f"ev{t}", name=f"ev{t}") for t in range(8)]
            cqT = [bB.tile([P, TOWN], MF, tag=f"cqT{c}", name=f"cqT{c}") for c in range(DC)]
            mergedT2 = [bB.tile([P, TOWN], MF, tag=f"mg2T{c}", name=f"mg2T{c}") for c in range(DC)]

            with tc.tile_pool(name="psB0", bufs=3, space="PSUM") as pp0:
                for t in range(8):
                    es = rB.tile([P, D], F32, tag="es")
                    nc.sync.dma_start(es[:], a["enc"][ts(t, P), :])
                    for c in range(DC):
                        tp = pp0.tile([P, P], F32, tag="tp")
                        nc.tensor.transpose(tp[:], es[:, ts(c, P)], ident[:])
                        nc.scalar.copy(encT[c][:, ts(t, P)], tp[:])

            with tc.tile_pool(name="wB", bufs=8) as wp, \
                 tc.tile_pool(name="psB1", bufs=3, space="PSUM") as pp1:
                proj_fm(ekT, encT, "eWkT", ebk_c, T, wpool=wp, pp=pp1)
                vproj(evsb, encT, "eWvT", ebv_r, wpool=wp, pp=pp1)
                proj_fm(cqT, x1T, "cWqT", cbq_c, TOWN, wpool=wp, pp=pp1)

            with tc.tile_pool(name="psS", bufs=3, space="PSUM") as sp, \
                 tc.tile_pool(name="psAV", bufs=2, space="PSUM") as avp, \
                 tc.tile_pool(name="psR", bufs=2, space="PSUM") as rp, \
                 tc.tile_pool(name="sbB", bufs=3) as sbp:
                attention(ekT, evsb, cqT, biasC, False, mergedT2,
                          (sp, avp, rp, sbp))

            with tc.tile_pool(name="wB2", bufs=4) as wp, \
                 tc.tile_pool(name="psB2", bufs=2, space="PSUM") as pp2, \
                 tc.tile_pool(name="psStat", bufs=1, space="PSUM") as statp, \
                 tc.tile_pool(name="psBC", bufs=2, space="PSUM") as bcp, \
                 tc.tile_pool(name="sbLN", bufs=3) as sbp:
                w_sb = []
                for c in range(DC):
                    w = wp.tile([P, D], MF, tag="w")
                    nc.sync.dma_start(w[:], a["Wo2T"][ts(c, P), :])
                    w_sb.append(w)
                ln_in = []
                for m in range(DC):
                    ps = pp2.tile([P, TOWN], F32, tag="proj")
                    for c in range(DC):
                        nc.tensor.matmul(ps[:], lhsT=_r(w_sb[c][:, ts(m, P)]),
                                         rhs=_r(mergedT2[c][:]),
                                         start=(c == 0), stop=(c == DC - 1))
                    li = sbp.tile([P, TOWN], MF, tag=f"li{m}", name=f"li{m}", bufs=1)
                    nc.vector.scalar_tensor_tensor(
                        li[:], in0=ps[:], scalar=bo2_c[:, m: m + 1],
                        in1=x1T[m][:], op0=OP.add, op1=OP.add)
                    ln_in.append(li)
                layernorm(ln_in, 2, x2T, (statp, bcp, sbp))

        # ================= block C: FFN, LN3 =================
        with tc.tile_pool(name="wC", bufs=4) as wp1, \
             tc.tile_pool(name="wC2", bufs=8) as wp2, \
             tc.tile_pool(name="hC", bufs=1) as hp, \
             tc.tile_pool(name="psC", bufs=3, space="PSUM") as pp, \
             tc.tile_pool(name="psStat", bufs=1, space="PSUM") as statp, \
             tc.tile_pool(name="psBC", bufs=2, space="PSUM") as bcp, \
             tc.tile_pool(name="sbC", bufs=3) as sbp:
            hT = [hp.tile([P, TOWN], MF, tag=f"hT{m}", name=f"hT{m}") for m in range(FFN // P)]
            proj_fm(hT, x2T, "W1T", b1_c, TOWN, func=AF.Relu, wpool=wp1, pp=pp)
            w_sb = []
            for c in range(FFN // P):
                w = wp2.tile([P, D], MF, tag="w2")
                nc.sync.dma_start(w[:], a["W2T"][ts(c, P), :])
                w_sb.append(w)
            ln_in = []
            for m in range(DC):
                ps = pp.tile([P, TOWN], F32, tag="proj")
                for c in range(FFN // P):
                    nc.tensor.matmul(ps[:], lhsT=_r(w_sb[c][:, ts(m, P)]),
                                     rhs=_r(hT[c][:]),
                                     start=(c == 0), stop=(c == FFN // P - 1))
                li = sbp.tile([P, TOWN], MF, tag=f"li{m}", name=f"li{m}", bufs=1)
                nc.vector.scalar_tensor_tensor(
                    li[:], in0=ps[:], scalar=b2_c[:, m: m + 1], in1=x2T[m][:],
                    op0=OP.add, op1=OP.add)
                ln_in.append(li)
            layernorm(ln_in, 3, x3T, (statp, bcp, sbp))

        # ================= block D: vocab projection =================
        with tc.tile_pool(name="wD", bufs=12) as wp, \
             tc.tile_pool(name="x3B", bufs=1) as xbp, \
             tc.tile_pool(name="stD", bufs=6) as stp, \
             tc.tile_pool(name="psD", bufs=2, space="PSUM") as pp:
            if WOUT_DT is BF16:
                x3v = [xbp.tile([P, TOWN], BF16, tag=f"x3B{c}", name=f"x3B{c}")
                       for c in range(DC)]
                for c in range(DC):
                    nc.scalar.copy(x3v[c][:], x3T[c][:])
            else:
                x3v = x3T
            for vg in range(NVG):
                w_sb = []
                for c in range(DC):
                    w = wp.tile([P, VG * VCH], WOUT_DT, tag="wo")
                    nc.sync.dma_start(
                        w[:], a["WoutT"][ts(c, P),
                                         vg * VG * VCH:(vg + 1) * VG * VCH])
                    w_sb.append(w)
                for t in range(TOWN // P):
                    ps = pp.tile([P, VG, 512], F32, tag="vps")
                    for j in range(VG):
                        for c in range(DC):
                            nc.tensor.matmul(
                                ps[:, j, 0:VCH],
                                lhsT=x3v[c][:, ts(t, P)],
                                rhs=w_sb[c][:, ts(j, VCH)],
                                start=(c == 0), stop=(c == DC - 1))
                    stage = stp.tile([P, VG * VCH], OUT_DT, tag="stage")
                    st3 = stage[:].rearrange("p (j e) -> p j e", e=VCH)
                    if t % 2 == 0:
                        nc.scalar.copy(st3, ps[:, :, 0:VCH])
                    else:
                        nc.vector.tensor_copy(st3, ps[:, :, 0:VCH])
                    nc.sync.dma_start(
                        a["out"][ts(t, P), vg * VG * VCH:(vg + 1) * VG * VCH],
                        stage[:])


# --------------------------------------------------------------------------
# host-side input preparation
# --------------------------------------------------------------------------

def _pos_encoding_np(t, d):
    pos = np.arange(t, dtype=np.float32)[:, None]
    freqs = 1.0 / (10000.0 ** (np.arange(0, d, 2, dtype=np.float32) / d))
    pe = np.zeros((t, d), np.float32)
    pe[:, 0::2] = np.sin(pos * freqs)
    pe[:, 1::2] = np.cos(pos * freqs)
    return pe


def _col_pack(b):
    """[n] -> [P, n//P] with element (p, c) = b[c*P + p]."""
    b = np.asarray(b, np.float32)
    return np.ascontiguousarray(b.reshape(-1, P).T)


def prep_in_maps(inputs):
    gi = lambda n: np.asarray(inputs[n])
    tokens = gi("tokens").astype(np.int32)                      # [4, 1024]
    enc_all = np.ascontiguousarray(gi("enc_embeddings").astype(np.float32))
    enc_pad = gi("enc_pad_mask").astype(bool)
    emb = np.ascontiguousarray(gi("emb").astype(np.float32))

    shared = {"emb": emb}
    for nm in ("Wq", "Wk", "Wv", "Wo1", "cWq", "eWk", "eWv", "Wo2", "W1", "W2",
               "Wout"):
        shared[nm + "T"] = np.ascontiguousarray(
            gi(nm).astype(np.float32).T)
    if WOUT_DT is BF16:
        import ml_dtypes
        shared["WoutT"] = shared["WoutT"].astype(ml_dtypes.bfloat16)
    for nm, src in (("bq", "bq"), ("bk", "bk"), ("bo1", "bo1"), ("cbq", "cbq"),
                    ("ebk", "ebk"), ("bo2", "bo2"), ("b2", "b2"), ("b1", "b1")):
        shared[nm + "_c"] = _col_pack(gi(src))
    shared["bv_r"] = gi("bv").astype(np.float32).reshape(1, D)
    shared["ebv_r"] = gi("ebv").astype(np.float32).reshape(1, D)
    for i, (g, b) in ((1, ("g1", "be1")), (2, ("g3", "be3")), (3, ("g2", "be2"))):
        shared[f"gc{i}"] = _col_pack(gi(g))
        shared[f"bc{i}"] = _col_pack(gi(b))

    # causal diagonal-block masks for key chunks 4..7 (slot space)
    kk = np.arange(P)[:, None]
    qq = np.arange(TOWN)[None, :]
    masks = np.zeros((P, 4, TOWN), np.float32)
    for j in range(4):
        masks[:, j, :] = np.where((j * P + kk) > qq, NEG, 0.0)
    shared["masks"] = masks

    pe = _pos_encoding_np(T, D)

    in_maps = []
    for core in range(8):
        b, hf = core // 2, core % 2
        own = tokens[b, hf * 512:(hf + 1) * 512]
        idx_full = np.concatenate([tokens[b, :512], own])        # [1024]
        pe_slots = np.concatenate([pe[:512], pe[hf * 512:(hf + 1) * 512]], axis=0)
        peT = np.ascontiguousarray(
            pe_slots.T.reshape(DC, P, T, order="C"))             # careful below
        # pe_slots.T is [D, T]; reshape to [DC, P, T] splits D into chunks
        biasS = np.where(idx_full == PAD_ID, NEG, 0.0).astype(np.float32)
        if hf == 0:
            biasS[:512] = NEG                                    # no prefix half
        biasC = np.where(enc_pad[b], NEG, 0.0).astype(np.float32)
        m = dict(shared)
        m["idx"] = np.ascontiguousarray(idx_full.reshape(T, 1))
        m["peT"] = peT
        m["enc"] = np.ascontiguousarray(enc_all[b])
        m["biasS"] = np.ascontiguousarray(biasS.reshape(8, P).T)
        m["biasC"] = np.ascontiguousarray(biasC.reshape(8, P).T)
        in_maps.append(m)
    return in_maps


def assemble(results, inputs):
    full = np.empty((4, 1024, V), np.float32)
    for core in range(8):
        b, hf = core // 2, core % 2
        full[b, hf * 512:(hf + 1) * 512] = np.asarray(
            results[core]["out"]).astype(np.float32)
    bout = np.asarray(inputs["bout"], np.float32)
    if np.any(bout):
        full += bout[None, None, :]
    return full


# --------------------------------------------------------------------------
# public entry point
# --------------------------------------------------------------------------

def kernel(**inputs):
    from concourse.bass_utils import run_bass_kernel_spmd
    nc = build_module()
    in_maps = prep_in_maps(inputs)
    res = run_bass_kernel_spmd(nc, in_maps, core_ids=list(range(8)))
    return assemble(res.results, inputs)


if __name__ == "__main__":
    nc = build_module()
    print("built ok")
